# revision 1
# baseline (speedup 1.0000x reference)
"""Sliding-window (banded) attention for nn_AttLayer on 8 Trainium2 NeuronCores.

Reference computation (per window-block n of 512 positions, 64 blocks over L=32768):
  q/k/v = 1x1-conv projections of x1 (512ch -> 256ch)
  energy[l, m] = (q_block[:, l] . k_window[:, m]) / 16   over a 1024-wide window
  attn = softmax(energy + log(band_mask + 1e-6)) * band_mask
  out  = relu(v_window @ attn^T) -> 1x1-conv (256 -> 512) + bias, masked

Sharding: 64 blocks split contiguously across 8 cores (8 blocks each). Each core
gets a zero-padded halo slice of x1 and computes its 4096 output columns.

Kernel strategy (per core, SPMD — all per-core variation is in the data):
  - Projections computed on PE with float32r (fp32 with 12-bit significand;
    matmul is exact for pre-rounded inputs). q/k natural layout (c on
    partitions), v projected directly TRANSPOSED (positions on partitions) so
    the attention AV matmul needs no transposes.
  - energy computed transposed: energyT[m, l] = k_chunk^T q  (PE), only over
    the ~62% of 128x128 tiles that intersect the band (padded to N>=256).
  - Band masking (MASK_MODE="gpsimd"): affine_select on the otherwise-idle
    GPSIMD engine zeroes out-of-band exp values; sequence-edge padding is
    handled with per-core 0/1 data vectors so the program stays SPMD.
    (MASK_MODE="pe" alternative: additive ln(1e-6) mask preloaded into PSUM
    via a bf16 identity matmul, exactly matching the reference's +1e-6 terms.)
  - exp on ScalarE (free scale=1/16), denominators via an all-ones f32r matmul
    (column sums land replicated across partitions), reciprocal on VectorE.
  - AV + output projection on PE; normalization fused into the PSUM->SBUF
    eviction with scalar_tensor_tensor. Final bias/mask applied on host.
  - Blocks are software-pipelined (block b's colsum/AV/outproj emitted after
    block b+1's energy+exp) so PE never waits on the softmax chain, and the
    k/q/vT projection groups are interleaved INTO the block stream: the
    k-projection alone would consume x1 above the HBM wire rate, so each
    fresh-x-hungry k group is followed by work on already-resident data.
  - Halo reuse between the two halves: half 1's first four vT tiles alias
    half 0's last four (identical x1 columns), and half 0's k overlap is
    stashed via an SBUF->SBUF DMA so half 1 skips its first k-group.
"""

import numpy as np

NCORES = 8
L = 32768
CIN = 512
C = 256
BL = 512
HALF = 256
LC = L // NCORES              # 4096 positions per core
HALO = LC + 2 * HALF          # 4608
NBH = 2                       # halves per core
LH = LC // NBH                # 2048 positions per half
KSPAN = LH + 2 * HALF         # 2560 k/v positions per half
BPH = 4                       # blocks per half
SCALE = 1.0 / 16.0
NEG = float(np.log(1e-6) / SCALE)   # additive raw-energy mask ~= -221.048

# Per m-chunk r' (8 chunks of the 1024-wide window): padded valid l-interval
# (lo, width) within the block's 512 queries, all widths >= 256 for f32r speed.
INTERVALS = [
    (0, 256), (0, 256), (0, 384), (0, 512),
    (0, 512), (128, 384), (256, 256), (256, 256),
]
MOFF = np.cumsum([0] + [w for _, w in INTERVALS]).tolist()  # offsets into mT_int
MTOT = MOFF[-1]  # 2816
# accumulation order: r'=3 covers the full [0,512) so it goes first (start=True)
AVORDER = [3, 4, 2, 5, 1, 6, 0, 7]
# "pe": additive log-mask preloaded into PSUM via bf16 identity matmul.
# "gpsimd": band-mask applied post-exp with affine_select on the idle Pool
#           engine (masked terms become exact zeros); edges via per-core data.
MASK_MODE = "gpsimd"


def _round_f32r(x):
    # round-to-nearest into the f32r grid (fp32 with low 12 mantissa bits zero)
    b = np.ascontiguousarray(x, dtype=np.float32).view(np.uint32)
    return ((b + np.uint32(0x800)) & np.uint32(0xFFFFF000)).view(np.float32)


def _mask_tile(r, lo, w, all_pad):
    if all_pad:
        return np.full((128, w), NEG, dtype=np.float32)
    m = np.arange(128 * r, 128 * r + 128, dtype=np.int64)[:, None]
    l = np.arange(lo, lo + w, dtype=np.int64)[None, :]
    valid = (m - l >= 0) & (m - l <= BL - 1)
    return np.where(valid, 0.0, NEG).astype(np.float32)


def _build_program():
    import concourse.mybir as mybir
    from concourse import bacc
    from concourse.tile import TileContext

    F32 = mybir.dt.float32
    F32R = mybir.dt.float32r
    BF16 = mybir.dt.bfloat16
    Alu = mybir.AluOpType
    Act = mybir.ActivationFunctionType

    nc = bacc.Bacc()

    x1h_d = nc.dram_tensor("x1h", [CIN, HALO], F32R, kind="ExternalInput")
    wqT_d = nc.dram_tensor("wqT", [128, 4 * C], F32R, kind="ExternalInput")
    wkT_d = nc.dram_tensor("wkT", [128, 4 * C], F32R, kind="ExternalInput")
    wvT_d = nc.dram_tensor("wvT", [128, 4 * C], F32R, kind="ExternalInput")
    woT_d = nc.dram_tensor("woT", [C, CIN], F32R, kind="ExternalInput")
    bq_d = nc.dram_tensor("bq2", [2, 128, 1], F32, kind="ExternalInput")
    bk_d = nc.dram_tensor("bk2", [2, 128, 1], F32, kind="ExternalInput")
    bvr_d = nc.dram_tensor("bvr", [128, C], F32, kind="ExternalInput")
    ones_d = nc.dram_tensor("ones", [128, 128], F32R, kind="ExternalInput")
    if MASK_MODE == "pe":
        ident_d = nc.dram_tensor("ident", [128, 128], BF16, kind="ExternalInput")
        mint_d = nc.dram_tensor("mT_int", [128, MTOT], BF16, kind="ExternalInput")
        mfirst_d = nc.dram_tensor("mT_first", [128, 512], BF16, kind="ExternalInput")
        mlast_d = nc.dram_tensor("mT_last", [128, 512], BF16, kind="ExternalInput")
    else:
        padf_d = nc.dram_tensor("padf", [2, 128, 1], F32, kind="ExternalInput")
        padl_d = nc.dram_tensor("padl", [2, 128, 1], F32, kind="ExternalInput")
    out_d = nc.dram_tensor("out", [CIN, LC], F32, kind="ExternalOutput")

    with TileContext(nc) as tc:
        with (
            tc.tile_pool(name="consts", bufs=1) as consts,
            tc.tile_pool(name="xpool", bufs=1) as xpool,
            tc.tile_pool(name="qkv", bufs=1) as qkv,
            tc.tile_pool(name="ptp", bufs=2) as ptp,
            tc.tile_pool(name="sbo", bufs=4) as sbo,
            tc.tile_pool(name="pse", bufs=3, space="PSUM") as pse,
            tc.tile_pool(name="pss", bufs=1, space="PSUM") as pss,
            tc.tile_pool(name="psav", bufs=1, space="PSUM") as psav,
            tc.tile_pool(name="pso", bufs=2, space="PSUM") as pso,
        ):
            # warm the ACT exp table while DMAs stream in
            warm_sb = consts.tile([1, 8], F32)
            nc.vector.memset(warm_sb, 0.0)
            nc.scalar.activation(warm_sb, warm_sb, Act.Exp)

            # warm the PE clock gate (HAM) during the initial DMA wait:
            # dummy bf16 matmuls on memset data keep the array busy so the
            # first real projections run at the full 2.4 GHz
            warm_a = consts.tile([128, 128], BF16, name="warm_a")
            nc.vector.memset(warm_a, 1.0)
            warm_b = consts.tile([128, 512], BF16, name="warm_b")
            nc.vector.memset(warm_b, 1.0)
            for wi in range(5):
                warm_ps = pse.tile([128, 512], F32, tag="e", name=f"wps{wi}")
                nc.tensor.matmul(warm_ps, warm_a, warm_b, start=True, stop=True)

            # critical-path-first DMA order: the first PE work is the h=0
            # k-projection of columns [0:512), needing wkT/bk and x chunk 0;
            # pair (wkT[kc], x[kc]) so the accumulation group streams in
            wT_sb = {}
            wk_all = consts.tile([128, 4 * C], F32R, name="wk_all")
            nc.sync.dma_start(out=wk_all, in_=wkT_d.ap())
            x_sb_h0 = []
            for kc in range(4):
                wT_sb[("k", kc)] = wk_all[:, C * kc:C * (kc + 1)]
                tx = xpool.tile([128, KSPAN], F32R, tag=f"x{kc}", name=f"x{kc}_0")
                x_sb_h0.append(tx)
                nc.sync.dma_start(
                    out=tx[:, 0:512],
                    in_=x1h_d.ap()[128 * kc:128 * (kc + 1), 0:512],
                )
            bk_sb = []
            for cc in range(2):
                tk = consts.tile([128, 1], F32, name=f"bk{cc}")
                nc.sync.dma_start(out=tk, in_=bk_d.ap()[cc])
                bk_sb.append(tk)
            def _x0_pair(ct):
                for kc in range(4):
                    nc.sync.dma_start(
                        out=x_sb_h0[kc][:, 512 * ct:512 * (ct + 2)],
                        in_=x1h_d.ap()[128 * kc:128 * (kc + 1),
                                       512 * ct:512 * (ct + 2)],
                    )

            _x0_pair(1)
            wq_all = consts.tile([128, 4 * C], F32R, name="wq_all")
            nc.sync.dma_start(out=wq_all, in_=wqT_d.ap())
            for kc in range(4):
                wT_sb[("q", kc)] = wq_all[:, C * kc:C * (kc + 1)]
            bq_sb = []
            for cc in range(2):
                tq = consts.tile([128, 1], F32, name=f"bq{cc}")
                nc.sync.dma_start(out=tq, in_=bq_d.ap()[cc])
                bq_sb.append(tq)
            wv_all = consts.tile([128, 4 * C], F32R, name="wv_all")
            nc.sync.dma_start(out=wv_all, in_=wvT_d.ap())
            for kc in range(4):
                wT_sb[("v", kc)] = wv_all[:, C * kc:C * (kc + 1)]
            bvrep_sb = consts.tile([128, C], F32)
            nc.sync.dma_start(out=bvrep_sb, in_=bvr_d.ap())
            _x0_pair(3)

            ones_sb = consts.tile([128, 128], F32R)
            nc.sync.dma_start(out=ones_sb, in_=ones_d.ap())
            if MASK_MODE == "pe":
                ident_sb = consts.tile([128, 128], BF16)
                nc.sync.dma_start(out=ident_sb, in_=ident_d.ap())
                mint_sb = consts.tile([128, MTOT], BF16)
                nc.sync.dma_start(out=mint_sb, in_=mint_d.ap())
                mfirst_sb = consts.tile([128, 512], BF16)
                nc.sync.dma_start(out=mfirst_sb, in_=mfirst_d.ap())
                mlast_sb = consts.tile([128, 512], BF16)
                nc.sync.dma_start(out=mlast_sb, in_=mlast_d.ap())
            else:
                padf_sb, padl_sb = [], []
                for r in range(2):
                    tf = consts.tile([128, 1], F32, name=f"padf{r}")
                    nc.sync.dma_start(out=tf, in_=padf_d.ap()[r])
                    padf_sb.append(tf)
                    tl = consts.tile([128, 1], F32, name=f"padl{r}")
                    nc.sync.dma_start(out=tl, in_=padl_d.ap()[r])
                    padl_sb.append(tl)
            woT_sb = []
            for cc in range(2):
                t = consts.tile([128, CIN], F32R, name=f"woT{cc}")
                nc.sync.dma_start(out=t, in_=woT_d.ap()[128 * cc:128 * (cc + 1), :])
                woT_sb.append(t)

            for h in range(NBH):
                base = LH * h  # halo-coord start of this half's x1/k/v span
                if h == 0:
                    x_sb = x_sb_h0
                else:
                    x_sb = []
                    for kc in range(4):
                        t = xpool.tile([128, KSPAN], F32R, tag=f"x{kc}",
                                       name=f"x{kc}_{h}")
                        x_sb.append(t)
                # split per 512-column chunk so projections start while the
                # rest of the slice streams in (all h=0 chunks issued up top)
                if h > 0:
                    for kc in range(4):
                        nc.sync.dma_start(
                            out=x_sb[kc][:, 256:1536],
                            in_=x1h_d.ap()[128 * kc:128 * (kc + 1),
                                           base + 256:base + 1536],
                        )
                    for kc in range(4):
                        nc.sync.dma_start(
                            out=x_sb[kc][:, 1536:2560],
                            in_=x1h_d.ap()[128 * kc:128 * (kc + 1),
                                           base + 1536:base + 2560],
                        )

                # ---- projections ----
                q_sb, k_sb = [], []
                for cc in range(2):
                    q_sb.append(qkv.tile([128, LH], F32R, tag=f"q{cc}", name=f"q{cc}_{h}"))
                    k_sb.append(qkv.tile([128, KSPAN], F32R, tag=f"k{cc}", name=f"k{cc}_{h}"))
                # projection group emitters; actual emission is interleaved
                # with the attention blocks below so the k-projection's burst
                # demand for fresh x chunks never outruns the DMA wire rate
                def k_group(mt):
                    for cc in range(2):
                        csl = slice(128 * cc, 128 * (cc + 1))
                        ps = pse.tile([128, 512], F32, tag="e",
                                      name=f"psk{h}{cc}{mt}")
                        for kc in range(4):
                            nc.tensor.matmul(
                                ps, wT_sb[("k", kc)][:, csl],
                                x_sb[kc][:, 512 * mt:512 * (mt + 1)],
                                start=(kc == 0), stop=(kc == 3),
                            )
                        nc.vector.tensor_scalar_add(
                            k_sb[cc][:, 512 * mt:512 * (mt + 1)], ps, bk_sb[cc]
                        )

                def q_group(lt):
                    for cc in range(2):
                        csl = slice(128 * cc, 128 * (cc + 1))
                        ps = pse.tile([128, 512], F32, tag="e",
                                      name=f"psq{h}{cc}{lt}")
                        for kc in range(4):
                            nc.tensor.matmul(
                                ps, wT_sb[("q", kc)][:, csl],
                                x_sb[kc][:, HALF + 512 * lt: HALF + 512 * (lt + 1)],
                                start=(kc == 0), stop=(kc == 3),
                            )
                        nc.vector.tensor_scalar_add(
                            q_sb[cc][:, 512 * lt:512 * (lt + 1)], ps, bq_sb[cc]
                        )

                vT_sb = [None] * (KSPAN // 128)
                if h > 0:
                    # halo reuse: this half's m=0..3 v-chunks cover the same
                    # x1 columns as the previous half's m=16..19 — alias them
                    for mt in range(4):
                        vT_sb[mt] = prev_vT[16 + mt]

                def vT_group(mts):
                    for mt in mts:
                        ps = pso.tile([128, C], F32, tag="o", name=f"psv{h}{mt}")
                        for kc in range(4):
                            nc.tensor.matmul(
                                ps, x_sb[kc][:, 128 * mt:128 * (mt + 1)],
                                wT_sb[("v", kc)], start=(kc == 0), stop=(kc == 3),
                            )
                        t = qkv.tile([128, C], F32R, tag=f"v{mt}", name=f"vT{mt}_{h}")
                        # eviction with the (per-free-element) v bias folded in
                        nc.vector.tensor_tensor(t, ps, bvrep_sb, op=Alu.add)
                        vT_sb[mt] = t

                # ---- attention blocks (software-pipelined: block b's
                # colsum/AV/outproj are emitted after block b+1's energy+exp
                # so PE never waits on the ACT/Pool softmax chain) ----
                def emit_energy(h, b, k_sb=k_sb, q_sb=q_sb, vT_sb=vT_sb,
                                kh=(kh_prev if h > 0 else None)):
                    woff = 512 * b   # window start in k/vT coords
                    first_blk = (h == 0 and b == 0)
                    last_blk = (h == NBH - 1 and b == BPH - 1)
                    pt = {}
                    for r in AVORDER:
                        lo, w = INTERVALS[r]
                        ps_e = pse.tile([128, w], F32, tag="e", name=f"pse{h}{b}{r}")
                        if MASK_MODE == "pe":
                            if first_blk and r < 2:
                                msrc = mfirst_sb[:, 256 * r:256 * r + w]
                            elif last_blk and r >= 6:
                                msrc = mlast_sb[:, 256 * (r - 6):256 * (r - 6) + w]
                            else:
                                msrc = mint_sb[:, MOFF[r]:MOFF[r] + w]
                            nc.tensor.matmul(ps_e, ident_sb, msrc, start=True,
                                             stop=False, skip_group_check=True)
                        for cc in range(2):
                            if kh is not None and b == 0 and r < 4:
                                klhs = kh[cc][:, 128 * r:128 * (r + 1)]
                            else:
                                klhs = k_sb[cc][:, woff + 128 * r:
                                                woff + 128 * (r + 1)]
                            nc.tensor.matmul(
                                ps_e, klhs,
                                q_sb[cc][:, 512 * b + lo: 512 * b + lo + w],
                                start=(MASK_MODE != "pe" and cc == 0),
                                stop=(cc == 1), skip_group_check=True,
                            )
                        t = ptp.tile([128, w], F32R, tag=f"pt{r}", name=f"pt{r}_{h}{b}")
                        nc.scalar.activation(t, ps_e, Act.Exp, scale=SCALE)
                        if MASK_MODE == "gpsimd":
                            # zero outside the band: one affine compare per tile
                            # (lower bound bites for r<=3, upper for r>=4)
                            if r <= 3:
                                nc.gpsimd.affine_select(
                                    out=t, in_=t, compare_op=Alu.is_ge, fill=0.0,
                                    base=128 * r - lo, channel_multiplier=1,
                                    pattern=[[-1, w]],
                                )
                            else:
                                # valid iff (128r+m')-l <= 511, recast as
                                # (511-128r+lo) - m' + j >= 0 (is_ge only)
                                nc.gpsimd.affine_select(
                                    out=t, in_=t, compare_op=Alu.is_ge, fill=0.0,
                                    base=(BL - 1) - 128 * r + lo,
                                    channel_multiplier=-1,
                                    pattern=[[1, w]],
                                )
                            if first_blk and r < 2:
                                nc.vector.tensor_scalar_mul(t, t, padf_sb[r])
                            elif last_blk and r >= 6:
                                nc.vector.tensor_scalar_mul(t, t, padl_sb[r - 6])
                        pt[r] = t
                    return (h, b, pt, vT_sb)

                def emit_tail(ctx):
                    h, b, pt, vT_l = ctx
                    ps_s = pss.tile([128, 512], F32, tag="s", name=f"pss{h}{b}")
                    for i, r in enumerate(AVORDER):
                        lo, w = INTERVALS[r]
                        nc.tensor.matmul(
                            ps_s[:, lo:lo + w], ones_sb, pt[r],
                            start=(i == 0), stop=(i == 7), skip_group_check=True,
                        )
                    recip = sbo.tile([128, 512], F32, tag="recip", name=f"rc{h}{b}")
                    nc.vector.reciprocal(recip, ps_s)

                    ps_av = []
                    for cc in range(2):
                        ps_av.append(psav.tile([128, 512], F32, tag=f"av{cc}",
                                               name=f"psav{h}{b}{cc}"))
                    for i, r in enumerate(AVORDER):
                        lo, w = INTERVALS[r]
                        for cc in range(2):
                            nc.tensor.matmul(
                                ps_av[cc][:, lo:lo + w],
                                vT_l[4 * b + r][:, 128 * cc:128 * (cc + 1)], pt[r],
                                start=(i == 0), stop=(i == 7), skip_group_check=True,
                            )
                    relu_sb = []
                    for cc in range(2):
                        t = sbo.tile([128, 512], F32R, tag=f"relu{cc}",
                                     name=f"relu{h}{b}{cc}")
                        nc.vector.tensor_scalar_max(t, ps_av[cc], 0.0)
                        relu_sb.append(t)

                    for oc in range(4):
                        ps_o = pso.tile([128, 512], F32, tag="o", name=f"pso{h}{b}{oc}")
                        for cc in range(2):
                            nc.tensor.matmul(
                                ps_o, woT_sb[cc][:, 128 * oc:128 * (oc + 1)],
                                relu_sb[cc], start=(cc == 0), stop=(cc == 1),
                            )
                        o_sb = sbo.tile([128, 512], F32, tag="osb", name=f"o{h}{b}{oc}")
                        nc.vector.scalar_tensor_tensor(
                            o_sb, ps_o, 0.0, recip, op0=Alu.bypass, op1=Alu.mult
                        )
                        nc.sync.dma_start(
                            out=out_d.ap()[128 * oc:128 * (oc + 1),
                                           LH * h + 512 * b: LH * h + 512 * (b + 1)],
                            in_=o_sb,
                        )

                pending = []
                for b in range(BPH):
                    if b == 0:
                        if h == 0:
                            k_group(0)
                        k_group(1)
                        q_group(0)
                        vT_group(range(0, 8) if h == 0 else range(4, 8))
                    else:
                        k_group(b + 1)
                        q_group(b)
                        vT_group(range(4 * b + 4, 4 * b + 8))
                    pending.append(emit_energy(h, b))
                    if len(pending) > 1:
                        emit_tail(pending.pop(0))
                # flush before the next half's projections overwrite q/k/vT
                for ctx in pending:
                    emit_tail(ctx)
                prev_vT = vT_sb
                if h == 0:
                    # stash the k halo overlap for the next half (SBUF->SBUF
                    # DMA, off-engine); half1's block 0 reads it directly
                    kh_prev = []
                    for cc in range(2):
                        tkh = qkv.tile([128, 512], F32R, tag=f"kh{cc}",
                                       name=f"kh{cc}")
                        nc.sync.dma_start(out=tkh,
                                          in_=k_sb[cc][:, LH:LH + 512])
                        kh_prev.append(tkh)
    nc.compile()
    return nc


_NC_CACHE = {}


def _get_nc():
    if "nc" not in _NC_CACHE:
        _NC_CACHE["nc"] = _build_program()
    return _NC_CACHE["nc"]


def make_in_maps(x1, mask, Wq, bq, Wk, bk, Wv, bv, Wo, bo):
    x1 = np.asarray(x1, dtype=np.float32).reshape(CIN, L)
    def _ileave(w):
        # (512, 256) -> (128, 4*256): chunk kc at columns [256*kc, 256*(kc+1))
        wt = _round_f32r(np.asarray(w, np.float32).T)
        return np.ascontiguousarray(
            wt.reshape(4, 128, C).transpose(1, 0, 2).reshape(128, 4 * C))
    wqT = _ileave(Wq)
    wkT = _ileave(Wk)
    wvT = _ileave(Wv)
    woT = _round_f32r(np.asarray(Wo, np.float32).T)
    bq2 = np.asarray(bq, np.float32).reshape(2, 128, 1)
    bk2 = np.asarray(bk, np.float32).reshape(2, 128, 1)
    bvr = np.ascontiguousarray(
        np.broadcast_to(np.asarray(bv, np.float32).reshape(1, C), (128, C))
    )
    ones = np.ones((128, 128), np.float32)
    ident = np.eye(128, dtype=np.float32)

    try:
        import ml_dtypes
        bf16 = ml_dtypes.bfloat16
    except ImportError:  # pragma: no cover
        import jax.numpy as jnp
        bf16 = jnp.bfloat16

    if MASK_MODE == "pe":
        mint = np.concatenate(
            [_mask_tile(r, lo, w, False) for r, (lo, w) in enumerate(INTERVALS)], axis=1
        ).astype(bf16)
        m_first_int = mint[:, :512].copy()
        m_last_int = mint[:, MOFF[6]:MTOT].copy()
        m_all_pad = np.full((128, 512), NEG, np.float32).astype(bf16)

    pad_ones = np.ones((2, 128, 1), np.float32)
    pad_zeros = np.zeros((2, 128, 1), np.float32)

    in_maps = []
    for c in range(NCORES):
        g0 = LC * c - HALF
        x1h = np.zeros((CIN, HALO), np.float32)
        s0, s1 = max(g0, 0), min(g0 + HALO, L)
        x1h[:, s0 - g0:s1 - g0] = x1[:, s0:s1]
        m = {
            "x1h": _round_f32r(x1h),
            "wqT": wqT, "wkT": wkT, "wvT": wvT, "woT": woT,
            "bq2": bq2, "bk2": bk2, "bvr": bvr,
            "ones": ones,
        }
        if MASK_MODE == "pe":
            m["ident"] = ident.astype(bf16)
            m["mT_int"] = mint
            m["mT_first"] = m_all_pad if c == 0 else m_first_int
            m["mT_last"] = m_all_pad if c == NCORES - 1 else m_last_int
        else:
            m["padf"] = pad_zeros if c == 0 else pad_ones
            m["padl"] = pad_zeros if c == NCORES - 1 else pad_ones
        in_maps.append(m)
    return in_maps


def postprocess(results, mask, bo):
    cols = np.concatenate([results[c]["out"] for c in range(NCORES)], axis=1)
    out = cols[None] + np.asarray(bo, np.float32)[None, :, None]
    return (out * np.asarray(mask, np.float32)).astype(np.float32)


def kernel(x1, x2, mask, Wq, bq, Wk, bk, Wv, bv, Wo, bo, **_unused):
    from concourse.bass_utils import run_bass_kernel_spmd

    nc = _get_nc()
    in_maps = make_in_maps(x1, mask, Wq, bq, Wk, bk, Wv, bv, Wo, bo)
    res = run_bass_kernel_spmd(nc, in_maps, core_ids=list(range(NCORES)))
    return postprocess(res.results, mask, bo)



# revision 4
# speedup vs baseline: 1.0692x; 1.0692x over previous
"""Sliding-window (banded) attention for nn_AttLayer on 8 Trainium2 NeuronCores.

Reference computation (per window-block n of 512 positions, 64 blocks over L=32768):
  q/k/v = 1x1-conv projections of x1 (512ch -> 256ch)
  energy[l, m] = (q_block[:, l] . k_window[:, m]) / 16   over a 1024-wide window
  attn = softmax(energy + log(band_mask + 1e-6)) * band_mask
  out  = relu(v_window @ attn^T) -> 1x1-conv (256 -> 512) + bias, masked

Sharding: 64 blocks split contiguously across 8 cores (8 blocks each). Each core
gets a zero-padded halo slice of x1 and computes its 4096 output columns.

Kernel strategy (per core, SPMD — all per-core variation is in the data):
  - Projections on PE in fp8e4 DoubleRow perf mode (0.5 cycles/row, two
    128-channel contraction tiles per instruction -> 4x MAC throughput).
    x1 and the projection weights are split host-side into e4m3 hi/lo pairs
    (W scaled by 64 to center the fp8 range; the 64^2 folds into the exp
    scale and 1/64 into Wo). Three accumulation chains (hi*hi + lo*hi +
    hi*lo) recover ~bf16-level accuracy at 0.75x the f32r cycle cost.
  - Everything downstream runs in fp16 (1 cycle/row like f32r but with no
    >=256 moving-width requirement, half the SBUF/DMA bytes of f32, and a
    10-bit mantissa -- ~10x less quantization error than bf16).
  - energy computed transposed: energyT[m, l] = k_chunk^T q (PE), over the
    EXACT per-chunk band intervals (2560 of 4096 window cols per block).
  - Band masking: affine_select on the otherwise-idle Pool/GPSIMD engine
    zeroes out-of-band exp values; sequence-edge padding handled with
    per-core 0/1 data vectors so the program stays SPMD.
  - exp on ScalarE (scale 2^-16 folds away the fp8 weight scaling),
    denominators via an all-ones fp16 matmul (column sums land replicated
    across partitions), reciprocal on VectorE.
  - AV on PE (fp16); softmax normalization + relu fused into the one
    PSUM->SBUF eviction (scalar_tensor_tensor max+mult) which also keeps
    the fp16 relu tile in range. Output projection on PE; its eviction is
    a plain ACT copy (psum -> fp16). Final bias/mask applied on host.
  - Blocks are software-pipelined (block b's colsum/AV/outproj emitted after
    block b+1's energy+exp) so PE never waits on the softmax chain, and the
    k/q/vT projection groups are interleaved INTO the block stream so the
    fresh-x DMA demand stays below the HBM wire rate.
  - Halo reuse between the two halves: half 1's first four vT tiles alias
    half 0's last four (identical x1 columns), and half 0's k overlap is
    stashed via an SBUF->SBUF DMA so half 1 skips its first k-group.
"""

import numpy as np

NCORES = 8
L = 32768
CIN = 512
C = 256
BL = 512
HALF = 256
LC = L // NCORES              # 4096 positions per core
HALO = LC + 2 * HALF          # 4608
NBH = 2                       # halves per core
LH = LC // NBH                # 2048 positions per half
KSPAN = LH + 2 * HALF         # 2560 k/v positions per half
BPH = 4                       # blocks per half
WSCALE = 64.0                 # host-side fp8 scaling of Wq/Wk/Wv (and biases)
EXP_SCALE = (1.0 / 16.0) / (WSCALE * WSCALE)   # softmax scale / W-scaling^2

# Per m-chunk r (8 chunks of the 1024-wide window): EXACT valid l-interval
# (lo, width) within the block's 512 queries (fp16 has no min-width penalty).
INTERVALS = [
    (0, 128), (0, 256), (0, 384), (0, 512),
    (0, 512), (128, 384), (256, 256), (384, 128),
]
# accumulation order: r=3 covers the full [0,512) so it goes first (start=True)
AVORDER = [3, 4, 2, 5, 1, 6, 0, 7]


def _build_program():
    import concourse.mybir as mybir
    from concourse import bacc
    from concourse.tile import TileContext

    F32 = mybir.dt.float32
    F16 = mybir.dt.float16
    F8 = mybir.dt.float8e4
    BF16 = mybir.dt.bfloat16
    Alu = mybir.AluOpType
    Act = mybir.ActivationFunctionType
    PM = mybir.MatmulPerfMode.DoubleRow

    nc = bacc.Bacc()

    # x hi/lo fp8 halo slices; weights in DoubleRow pair layout
    # [c_in_within_chunk(128), pair p, row j, c_out] with global input channel
    # 128*(2p+j) + c_in.
    xh_d = nc.dram_tensor("xh", [CIN, HALO], F8, kind="ExternalInput")
    xl_d = nc.dram_tensor("xl", [CIN, HALO], F8, kind="ExternalInput")
    w_d = {}
    for kind in ("q", "k", "v"):
        for part in ("h", "l"):
            w_d[(kind, part)] = nc.dram_tensor(
                f"w{kind}{part}", [128, 2, 2, C], F8, kind="ExternalInput")
    woT_d = nc.dram_tensor("woT", [C, CIN], F16, kind="ExternalInput")
    bq_d = nc.dram_tensor("bq2", [2, 128, 1], F32, kind="ExternalInput")
    bk_d = nc.dram_tensor("bk2", [2, 128, 1], F32, kind="ExternalInput")
    bvr_d = nc.dram_tensor("bvr", [128, C], F32, kind="ExternalInput")
    ones_d = nc.dram_tensor("ones", [128, 128], F16, kind="ExternalInput")
    padf_d = nc.dram_tensor("padf", [2, 128, 1], F32, kind="ExternalInput")
    padl_d = nc.dram_tensor("padl", [2, 128, 1], F32, kind="ExternalInput")
    out_d = nc.dram_tensor("out", [CIN, LC], F16, kind="ExternalOutput")

    with TileContext(nc) as tc:
        with (
            tc.tile_pool(name="consts", bufs=1) as consts,
            tc.tile_pool(name="xpool", bufs=1) as xpool,
            tc.tile_pool(name="qkv", bufs=1) as qkv,
            tc.tile_pool(name="ptp", bufs=2) as ptp,
            tc.tile_pool(name="sbo", bufs=4) as sbo,
            tc.tile_pool(name="pse", bufs=3, space="PSUM") as pse,
            tc.tile_pool(name="pss", bufs=1, space="PSUM") as pss,
            tc.tile_pool(name="psav", bufs=1, space="PSUM") as psav,
            tc.tile_pool(name="pso", bufs=2, space="PSUM") as pso,
        ):
            # warm the ACT exp table while DMAs stream in
            warm_sb = consts.tile([1, 8], F32)
            nc.vector.memset(warm_sb, 0.0)
            nc.scalar.activation(warm_sb, warm_sb, Act.Exp)

            # warm the PE clock gate (HAM) during the initial DMA wait:
            # dummy bf16 matmuls on memset data keep the array busy so the
            # first real projections run at the full 2.4 GHz
            warm_a = consts.tile([128, 128], BF16, name="warm_a")
            nc.vector.memset(warm_a, 1.0)
            warm_b = consts.tile([128, 512], BF16, name="warm_b")
            nc.vector.memset(warm_b, 1.0)
            for wi in range(5):
                warm_ps = pse.tile([128, 512], F32, tag="e", name=f"wps{wi}")
                nc.tensor.matmul(warm_ps, warm_a, warm_b, start=True, stop=True)

            # critical-path-first DMA order: the first PE work is the h=0
            # k-projection of columns [0:512), needing wk and x chunk 0
            wT_sb = {}
            for part in ("h", "l"):
                t = consts.tile([128, 2, 2, C], F8, name=f"wk{part}")
                nc.sync.dma_start(out=t, in_=w_d[("k", part)].ap())
                wT_sb[("k", part)] = t
            # x pair tiles: [128, pair j, pos] per (pair p, hi/lo)
            x_sb_h0 = {}
            for p in range(2):
                for part in ("h", "l"):
                    x_sb_h0[(p, part)] = xpool.tile(
                        [128, 2, KSPAN], F8, tag=f"x{p}{part}", name=f"x{p}{part}_0")

            def _x_dma(x_sb, p, part, j, a, b, base):
                src = xh_d if part == "h" else xl_d
                g0 = 128 * (2 * p + j)
                nc.sync.dma_start(
                    out=x_sb[(p, part)][:, j, a:b],
                    in_=src.ap()[g0:g0 + 128, base + a:base + b],
                )

            def _x0_piece(ct, n=1):
                for p in range(2):
                    for j in range(2):
                        for part in ("h", "l"):
                            _x_dma(x_sb_h0, p, part, j, 512 * ct, 512 * (ct + n), 0)

            _x0_piece(0)
            bk_sb = []
            for cc in range(2):
                tk = consts.tile([128, 1], F32, name=f"bk{cc}")
                nc.sync.dma_start(out=tk, in_=bk_d.ap()[cc])
                bk_sb.append(tk)
            _x0_piece(1, 2)
            for part in ("h", "l"):
                t = consts.tile([128, 2, 2, C], F8, name=f"wq{part}")
                nc.sync.dma_start(out=t, in_=w_d[("q", part)].ap())
                wT_sb[("q", part)] = t
            bq_sb = []
            for cc in range(2):
                tq = consts.tile([128, 1], F32, name=f"bq{cc}")
                nc.sync.dma_start(out=tq, in_=bq_d.ap()[cc])
                bq_sb.append(tq)
            for part in ("h", "l"):
                t = consts.tile([128, 2, 2, C], F8, name=f"wv{part}")
                nc.sync.dma_start(out=t, in_=w_d[("v", part)].ap())
                wT_sb[("v", part)] = t
            bvrep_sb = consts.tile([128, C], F32)
            nc.sync.dma_start(out=bvrep_sb, in_=bvr_d.ap())
            _x0_piece(3, 2)

            ones_sb = consts.tile([128, 128], F16)
            nc.sync.dma_start(out=ones_sb, in_=ones_d.ap())
            padf_sb, padl_sb = [], []
            for r in range(2):
                tf = consts.tile([128, 1], F32, name=f"padf{r}")
                nc.sync.dma_start(out=tf, in_=padf_d.ap()[r])
                padf_sb.append(tf)
                tl = consts.tile([128, 1], F32, name=f"padl{r}")
                nc.sync.dma_start(out=tl, in_=padl_d.ap()[r])
                padl_sb.append(tl)
            woT_sb = []
            for cc in range(2):
                t = consts.tile([128, CIN], F16, name=f"woT{cc}")
                nc.sync.dma_start(out=t, in_=woT_d.ap()[128 * cc:128 * (cc + 1), :])
                woT_sb.append(t)

            for h in range(NBH):
                base = LH * h  # halo-coord start of this half's x1/k/v span
                if h == 0:
                    x_sb = x_sb_h0
                else:
                    x_sb = {}
                    for p in range(2):
                        for part in ("h", "l"):
                            x_sb[(p, part)] = xpool.tile(
                                [128, 2, KSPAN], F8, tag=f"x{p}{part}",
                                name=f"x{p}{part}_{h}")
                    # split per chunk so projections start while the rest of
                    # the slice streams in (all h=0 pieces issued up top)
                    for p in range(2):
                        for j in range(2):
                            for part in ("h", "l"):
                                _x_dma(x_sb, p, part, j, 256, 1536, base)
                    for p in range(2):
                        for j in range(2):
                            for part in ("h", "l"):
                                _x_dma(x_sb, p, part, j, 1536, 2560, base)

                # ---- projections (fp8 DoubleRow, 3 hi/lo chains) ----
                q_sb, k_sb = [], []
                for cc in range(2):
                    q_sb.append(qkv.tile([128, LH], F16, tag=f"q{cc}", name=f"q{cc}_{h}"))
                    k_sb.append(qkv.tile([128, KSPAN], F16, tag=f"k{cc}", name=f"k{cc}_{h}"))

                CHAINS = (("h", "h"), ("l", "h"), ("h", "l"))

                def _proj_psum(kind, cc, ps, x0):
                    # accumulate W^T x into ps[128, 512] over K=512 via
                    # 2 DoubleRow pair-steps x 3 chains x 2 col-halves
                    csl = slice(128 * cc, 128 * (cc + 1))
                    for half_i in range(2):
                        n0 = x0 + 256 * half_i
                        first = True
                        for p in range(2):
                            for (wp, xp) in CHAINS:
                                nc.tensor.matmul(
                                    ps[:, 256 * half_i:256 * (half_i + 1)],
                                    wT_sb[(kind, wp)][:, p, :, csl],
                                    x_sb[(p, xp)][:, :, n0:n0 + 256],
                                    start=first, stop=(p == 1 and (wp, xp) == CHAINS[-1]),
                                    perf_mode=PM, skip_group_check=True,
                                )
                                first = False

                def k_group(mt):
                    for cc in range(2):
                        ps = pse.tile([128, 512], F32, tag="e",
                                      name=f"psk{h}{cc}{mt}")
                        _proj_psum("k", cc, ps, 512 * mt)
                        nc.vector.tensor_scalar_add(
                            k_sb[cc][:, 512 * mt:512 * (mt + 1)], ps, bk_sb[cc]
                        )

                def q_group(lt):
                    for cc in range(2):
                        ps = pse.tile([128, 512], F32, tag="e",
                                      name=f"psq{h}{cc}{lt}")
                        _proj_psum("q", cc, ps, HALF + 512 * lt)
                        nc.vector.tensor_scalar_add(
                            q_sb[cc][:, 512 * lt:512 * (lt + 1)], ps, bq_sb[cc]
                        )

                vT_sb = [None] * (KSPAN // 128)
                if h > 0:
                    # halo reuse: this half's m=0..3 v-chunks cover the same
                    # x1 columns as the previous half's m=16..19 — alias them
                    for mt in range(4):
                        vT_sb[mt] = prev_vT[16 + mt]

                def vT_group(mts):
                    for mt in mts:
                        ps = pso.tile([128, C], F32, tag="o", name=f"psv{h}{mt}")
                        first = True
                        for p in range(2):
                            for (wp, xp) in CHAINS:
                                nc.tensor.matmul(
                                    ps,
                                    x_sb[(p, xp)][:, :, 128 * mt:128 * (mt + 1)],
                                    wT_sb[("v", wp)][:, p],
                                    start=first, stop=(p == 1 and (wp, xp) == CHAINS[-1]),
                                    perf_mode=PM, skip_group_check=True,
                                )
                                first = False
                        t = qkv.tile([128, C], F16, tag=f"v{mt}", name=f"vT{mt}_{h}")
                        # eviction with the (per-free-element) v bias folded in
                        nc.vector.tensor_tensor(t, ps, bvrep_sb, op=Alu.add)
                        vT_sb[mt] = t

                # ---- attention blocks (software-pipelined: block b's
                # colsum/AV/outproj are emitted after block b+1's energy+exp
                # so PE never waits on the ACT/Pool softmax chain) ----
                def emit_energy(h, b, k_sb=k_sb, q_sb=q_sb, vT_sb=vT_sb,
                                kh=(kh_prev if h > 0 else None)):
                    woff = 512 * b   # window start in k/vT coords
                    first_blk = (h == 0 and b == 0)
                    last_blk = (h == NBH - 1 and b == BPH - 1)
                    pt = {}
                    for r in AVORDER:
                        lo, w = INTERVALS[r]
                        ps_e = pse.tile([128, w], F32, tag="e", name=f"pse{h}{b}{r}")
                        for cc in range(2):
                            if kh is not None and b == 0 and r < 4:
                                klhs = kh[cc][:, 128 * r:128 * (r + 1)]
                            else:
                                klhs = k_sb[cc][:, woff + 128 * r:
                                                woff + 128 * (r + 1)]
                            nc.tensor.matmul(
                                ps_e, klhs,
                                q_sb[cc][:, 512 * b + lo: 512 * b + lo + w],
                                start=(cc == 0), stop=(cc == 1),
                                skip_group_check=True,
                            )
                        t = ptp.tile([128, w], F16, tag=f"pt{r}", name=f"pt{r}_{h}{b}")
                        nc.scalar.activation(t, ps_e, Act.Exp, scale=EXP_SCALE)
                        # zero outside the band: one affine compare per tile
                        # (lower bound bites for r<=3, upper for r>=4)
                        if r <= 3:
                            nc.gpsimd.affine_select(
                                out=t, in_=t, compare_op=Alu.is_ge, fill=0.0,
                                base=128 * r - lo, channel_multiplier=1,
                                pattern=[[-1, w]],
                            )
                        else:
                            # valid iff (128r+m')-l <= 511, recast as
                            # (511-128r+lo) - m' + j >= 0 (is_ge only)
                            nc.gpsimd.affine_select(
                                out=t, in_=t, compare_op=Alu.is_ge, fill=0.0,
                                base=(BL - 1) - 128 * r + lo,
                                channel_multiplier=-1,
                                pattern=[[1, w]],
                            )
                        if first_blk and r < 2:
                            nc.vector.tensor_scalar_mul(t, t, padf_sb[r])
                        elif last_blk and r >= 6:
                            nc.vector.tensor_scalar_mul(t, t, padl_sb[r - 6])
                        pt[r] = t
                    return (h, b, pt, vT_sb)

                def emit_tail(ctx):
                    h, b, pt, vT_l = ctx
                    ps_s = pss.tile([128, 512], F32, tag="s", name=f"pss{h}{b}")
                    for i, r in enumerate(AVORDER):
                        lo, w = INTERVALS[r]
                        nc.tensor.matmul(
                            ps_s[:, lo:lo + w], ones_sb, pt[r],
                            start=(i == 0), stop=(i == 7), skip_group_check=True,
                        )
                    recip = sbo.tile([128, 512], F32, tag="recip", name=f"rc{h}{b}")
                    nc.vector.reciprocal(recip, ps_s)

                    ps_av = []
                    for cc in range(2):
                        ps_av.append(psav.tile([128, 512], F32, tag=f"av{cc}",
                                               name=f"psav{h}{b}{cc}"))
                    for i, r in enumerate(AVORDER):
                        lo, w = INTERVALS[r]
                        for cc in range(2):
                            nc.tensor.matmul(
                                ps_av[cc][:, lo:lo + w],
                                vT_l[4 * b + r][:, 128 * cc:128 * (cc + 1)], pt[r],
                                start=(i == 0), stop=(i == 7), skip_group_check=True,
                            )
                    # normalization fused into the relu eviction: keeps the
                    # fp16 tile in range and shortens the output tail
                    relu_sb = []
                    for cc in range(2):
                        t = sbo.tile([128, 512], F16, tag=f"relu{cc}",
                                     name=f"relu{h}{b}{cc}")
                        nc.vector.scalar_tensor_tensor(
                            t, ps_av[cc], 0.0, recip, op0=Alu.max, op1=Alu.mult
                        )
                        relu_sb.append(t)

                    for oc in range(4):
                        ps_o = pso.tile([128, 512], F32, tag="o", name=f"pso{h}{b}{oc}")
                        for cc in range(2):
                            nc.tensor.matmul(
                                ps_o, woT_sb[cc][:, 128 * oc:128 * (oc + 1)],
                                relu_sb[cc], start=(cc == 0), stop=(cc == 1),
                            )
                        o_sb = sbo.tile([128, 512], F16, tag="osb", name=f"o{h}{b}{oc}")
                        nc.scalar.activation(o_sb, ps_o, Act.Copy)
                        nc.sync.dma_start(
                            out=out_d.ap()[128 * oc:128 * (oc + 1),
                                           LH * h + 512 * b: LH * h + 512 * (b + 1)],
                            in_=o_sb,
                        )

                pending = []
                for b in range(BPH):
                    if b == 0:
                        if h == 0:
                            k_group(0)
                        k_group(1)
                        q_group(0)
                        vT_group(range(0, 8) if h == 0 else range(4, 8))
                    else:
                        k_group(b + 1)
                        q_group(b)
                        vT_group(range(4 * b + 4, 4 * b + 8))
                    pending.append(emit_energy(h, b))
                    if len(pending) > 1:
                        emit_tail(pending.pop(0))
                # flush before the next half's projections overwrite q/k/vT
                for ctx in pending:
                    emit_tail(ctx)
                prev_vT = vT_sb
                if h == 0:
                    # stash the k halo overlap for the next half (SBUF->SBUF
                    # DMA, off-engine); half1's block 0 reads it directly
                    kh_prev = []
                    for cc in range(2):
                        tkh = qkv.tile([128, 512], F16, tag=f"kh{cc}",
                                       name=f"kh{cc}")
                        nc.sync.dma_start(out=tkh,
                                          in_=k_sb[cc][:, LH:LH + 512])
                        kh_prev.append(tkh)
    nc.compile()
    return nc


_NC_CACHE = {}


def _get_nc():
    if "nc" not in _NC_CACHE:
        _NC_CACHE["nc"] = _build_program()
    return _NC_CACHE["nc"]


def _f8():
    try:
        import ml_dtypes
        return ml_dtypes.float8_e4m3
    except ImportError:  # pragma: no cover
        import jax.numpy as jnp
        return jnp.float8_e4m3


def _split8(a):
    f8 = _f8()
    hi = np.asarray(a, np.float32).astype(f8)
    lo = (np.asarray(a, np.float32) - hi.astype(np.float32)).astype(f8)
    return hi, lo


def make_in_maps(x1, mask, Wq, bq, Wk, bk, Wv, bv, Wo, bo):
    x1 = np.asarray(x1, dtype=np.float32).reshape(CIN, L)

    def _pairs(w):
        # (C_out=256, C_in=512) -> hi/lo [128, 2, 2, C] DoubleRow pair layout:
        # [c_in_within, pair p, row j, c_out], global c_in = 128*(2p+j)+c_in
        ws = np.asarray(w, np.float32) * WSCALE
        hi, lo = _split8(ws.T)          # (512, 256)
        def lay(a):
            return np.ascontiguousarray(
                a.reshape(2, 2, 128, C).transpose(2, 0, 1, 3))
        return lay(hi), lay(lo)

    wqh, wql = _pairs(Wq)
    wkh, wkl = _pairs(Wk)
    wvh, wvl = _pairs(Wv)
    woT = (np.asarray(Wo, np.float32).T / WSCALE).astype(np.float16)
    bq2 = (np.asarray(bq, np.float32) * WSCALE).reshape(2, 128, 1)
    bk2 = (np.asarray(bk, np.float32) * WSCALE).reshape(2, 128, 1)
    bvr = np.ascontiguousarray(np.broadcast_to(
        (np.asarray(bv, np.float32) * WSCALE).reshape(1, C), (128, C)))
    ones = np.ones((128, 128), np.float16)

    pad_ones = np.ones((2, 128, 1), np.float32)
    pad_zeros = np.zeros((2, 128, 1), np.float32)

    in_maps = []
    for c in range(NCORES):
        g0 = LC * c - HALF
        x1h = np.zeros((CIN, HALO), np.float32)
        s0, s1 = max(g0, 0), min(g0 + HALO, L)
        x1h[:, s0 - g0:s1 - g0] = x1[:, s0:s1]
        xh, xl = _split8(x1h)
        m = {
            "xh": xh, "xl": xl,
            "wqh": wqh, "wql": wql, "wkh": wkh, "wkl": wkl,
            "wvh": wvh, "wvl": wvl, "woT": woT,
            "bq2": bq2, "bk2": bk2, "bvr": bvr,
            "ones": ones,
            "padf": pad_zeros if c == 0 else pad_ones,
            "padl": pad_zeros if c == NCORES - 1 else pad_ones,
        }
        in_maps.append(m)
    return in_maps


def postprocess(results, mask, bo):
    cols = np.concatenate(
        [np.asarray(results[c]["out"], np.float32) for c in range(NCORES)], axis=1)
    out = cols[None] + np.asarray(bo, np.float32)[None, :, None]
    return (out * np.asarray(mask, np.float32)).astype(np.float32)


def kernel(x1, x2, mask, Wq, bq, Wk, bk, Wv, bv, Wo, bo, **_unused):
    from concourse.bass_utils import run_bass_kernel_spmd

    nc = _get_nc()
    in_maps = make_in_maps(x1, mask, Wq, bq, Wk, bk, Wv, bv, Wo, bo)
    res = run_bass_kernel_spmd(nc, in_maps, core_ids=list(range(NCORES)))
    return postprocess(res.results, mask, bo)


# revision 12
# speedup vs baseline: 1.1610x; 1.0859x over previous
"""Sliding-window (banded) attention for nn_AttLayer on 8 Trainium2 NeuronCores.

Reference computation (per window-block n of 512 positions, 64 blocks over L=32768):
  q/k/v = 1x1-conv projections of x1 (512ch -> 256ch)
  energy[l, m] = (q_block[:, l] . k_window[:, m]) / 16   over a 1024-wide window
  attn = softmax(energy + log(band_mask + 1e-6)) * band_mask
  out  = relu(v_window @ attn^T) -> 1x1-conv (256 -> 512) + bias, masked

Sharding: 64 blocks split contiguously across 8 cores (8 blocks each). Each core
gets a zero-padded halo slice of x1 and computes its 4096 output columns.

Kernel strategy (per core, SPMD — all per-core variation is in the data):
  - Projections on PE in fp8e4 DoubleRow perf mode (0.5 cycles/row, two
    128-channel contraction tiles per instruction -> 4x MAC throughput).
    x1 and the projection weights are split host-side into e4m3 hi/lo pairs
    (W scaled by 64 to center the fp8 range; the 64^2 folds into the exp
    scale and 1/64 into Wo). Three accumulation chains (hi*hi + lo*hi +
    hi*lo) recover ~bf16-level accuracy at 0.75x the f32r cycle cost.
  - Everything downstream runs in fp16 (1 cycle/row like f32r but with no
    >=256 moving-width requirement, half the SBUF/DMA bytes of f32, and a
    10-bit mantissa -- ~10x less quantization error than bf16).
  - energy computed transposed: energyT[m, l] = k_chunk^T q (PE), over the
    EXACT per-chunk band intervals (2560 of 4096 window cols per block).
  - Band masking: affine_select on the otherwise-idle Pool/GPSIMD engine
    zeroes out-of-band exp values; sequence-edge padding handled with
    per-core 0/1 data vectors so the program stays SPMD.
  - exp on ScalarE (scale 2^-16 folds away the fp8 weight scaling),
    denominators via an all-ones fp16 matmul (column sums land replicated
    across partitions), reciprocal on VectorE.
  - AV on PE (fp16); softmax normalization + relu fused into the one
    PSUM->SBUF eviction (scalar_tensor_tensor max+mult) which also keeps
    the fp16 relu tile in range. Output projection on PE; its eviction is
    a plain ACT copy (psum -> fp16). Final bias/mask applied on host.
  - Blocks are software-pipelined (block b's colsum/AV/outproj emitted after
    block b+1's energy+exp) so PE never waits on the softmax chain, and the
    k/q/vT projection groups are interleaved INTO the block stream so the
    fresh-x DMA demand stays below the HBM wire rate.
  - Halo reuse between the two halves: half 1's first four vT tiles alias
    half 0's last four (identical x1 columns), and half 0's k overlap is
    stashed via an SBUF->SBUF DMA so half 1 skips its first k-group.
"""

import numpy as np

NCORES = 8
L = 32768
CIN = 512
C = 256
BL = 512
HALF = 256
LC = L // NCORES              # 4096 positions per core
HALO = LC + 2 * HALF          # 4608
NBH = 2                       # halves per core
LH = LC // NBH                # 2048 positions per half
KSPAN = LH + 2 * HALF         # 2560 k/v positions per half
BPH = 4                       # blocks per half
WSCALE = 64.0                 # host-side fp8 scaling of Wq/Wk/Wv (and biases)
EXP_SCALE = (1.0 / 16.0) / (WSCALE * WSCALE)   # softmax scale / W-scaling^2

# Per m-chunk r (8 chunks of the 1024-wide window): EXACT valid l-interval
# (lo, width) within the block's 512 queries (fp16 has no min-width penalty).
INTERVALS = [
    (0, 128), (0, 256), (0, 384), (0, 512),
    (0, 512), (128, 384), (256, 256), (384, 128),
]
# accumulation order: r=3 covers the full [0,512) so it goes first (start=True)
AVORDER = [3, 4, 2, 5, 1, 6, 0, 7]


def _build_program():
    import concourse.mybir as mybir
    from concourse import bacc
    from concourse.tile import TileContext

    F32 = mybir.dt.float32
    F16 = mybir.dt.float16
    F8 = mybir.dt.float8e4
    BF16 = mybir.dt.bfloat16
    Alu = mybir.AluOpType
    Act = mybir.ActivationFunctionType
    PM = mybir.MatmulPerfMode.DoubleRow

    nc = bacc.Bacc()

    # x hi/lo fp8 halo slice, hi and lo planes interleaved per channel so one
    # DMA fills both; weights in DoubleRow pair layout
    # [c_in_within_chunk(128), hi/lo, pair p, row j, c_out] with global input
    # channel 128*(2p+j) + c_in.
    xhl_d = nc.dram_tensor("xhl", [CIN, 2, HALO], F8, kind="ExternalInput")
    w_d = {}
    for kind in ("q", "k", "v"):
        w_d[kind] = nc.dram_tensor(
            f"w{kind}", [128, 2, 2, 2, C], F8, kind="ExternalInput")
    # f32 scalar blob: [bq0 bq1 bk0 bk1 padf0 padf1 padl0 padl1 | bvr(256)]
    cb32_d = nc.dram_tensor("cb32", [128, 264], F32, kind="ExternalInput")
    # f16 blob: [ones(128) | woT0(512) | woT1(512)]
    cb16_d = nc.dram_tensor("cb16", [128, 1152], F16, kind="ExternalInput")
    out_d = nc.dram_tensor("out", [CIN, LC], F16, kind="ExternalOutput")

    with TileContext(nc) as tc:
        with (
            tc.tile_pool(name="consts", bufs=1) as consts,
            tc.tile_pool(name="xpool", bufs=2) as xpool,
            tc.tile_pool(name="qkv", bufs=1) as qkv,
            tc.tile_pool(name="ptp", bufs=2) as ptp,
            tc.tile_pool(name="sbo", bufs=4) as sbo,
            tc.tile_pool(name="pse", bufs=3, space="PSUM") as pse,
            tc.tile_pool(name="pss", bufs=1, space="PSUM") as pss,
            tc.tile_pool(name="psav", bufs=1, space="PSUM") as psav,
            tc.tile_pool(name="pso", bufs=2, space="PSUM") as pso,
        ):
            # warm the ACT exp table while DMAs stream in
            warm_sb = consts.tile([1, 8], F32)
            nc.vector.memset(warm_sb, 0.0)
            nc.scalar.activation(warm_sb, warm_sb, Act.Exp)

            # warm the PE clock gate (HAM) during the initial DMA wait:
            # dummy bf16 matmuls on memset data keep the array busy so the
            # first real projections run at the full 2.4 GHz
            warm_a = consts.tile([128, 128], BF16, name="warm_a")
            nc.vector.memset(warm_a, 1.0)
            warm_b = consts.tile([128, 512], BF16, name="warm_b")
            nc.vector.memset(warm_b, 1.0)
            for wi in range(5):
                warm_ps = pse.tile([128, 512], F32, tag="e", name=f"wps{wi}")
                nc.tensor.matmul(warm_ps, warm_a, warm_b, start=True, stop=True)

            # critical-path-first DMA order: the first PE work is the h=0
            # k-projection of columns [0:512), needing wk and x chunk 0
            wT_sb = {}
            t = consts.tile([128, 2, 2, 2, C], F8, name="wk")
            nc.sync.dma_start(out=t, in_=w_d["k"].ap())
            wT_sb["k"] = t
            # x pair tiles: [128, hi/lo, row j, pos] per pair p
            x_sb_h0 = {}
            for p in range(2):
                x_sb_h0[p] = xpool.tile(
                    [128, 2, 2, KSPAN], F8, tag=f"x{p}", name=f"x{p}_0")

            def _x_dma(x_sb, p, j, a, b, base):
                g0 = 128 * (2 * p + j)
                nc.sync.dma_start(
                    out=x_sb[p][:, :, j, a:b],
                    in_=xhl_d.ap()[g0:g0 + 128, :, base + a:base + b],
                )

            def _x0_piece(ct, n=1):
                for p in range(2):
                    for j in range(2):
                        _x_dma(x_sb_h0, p, j, 512 * ct, 512 * (ct + n), 0)

            _x0_piece(0)
            cb32_sb = consts.tile([128, 264], F32, name="cb32")
            nc.sync.dma_start(out=cb32_sb, in_=cb32_d.ap())
            bq_sb = [cb32_sb[:, 0:1], cb32_sb[:, 1:2]]
            bk_sb = [cb32_sb[:, 2:3], cb32_sb[:, 3:4]]
            padf_sb = [cb32_sb[:, 4:5], cb32_sb[:, 5:6]]
            padl_sb = [cb32_sb[:, 6:7], cb32_sb[:, 7:8]]
            bvrep_sb = cb32_sb[:, 8:264]
            _x0_piece(1, 2)
            t = consts.tile([128, 2, 2, 2, C], F8, name="wq")
            nc.sync.dma_start(out=t, in_=w_d["q"].ap())
            wT_sb["q"] = t
            t = consts.tile([128, 2, 2, 2, C], F8, name="wv")
            nc.sync.dma_start(out=t, in_=w_d["v"].ap())
            wT_sb["v"] = t
            _x0_piece(3, 2)

            cb16_sb = consts.tile([128, 1152], F16, name="cb16")
            nc.sync.dma_start(out=cb16_sb, in_=cb16_d.ap())
            ones_sb = cb16_sb[:, 0:128]
            woT_sb = [cb16_sb[:, 128:640], cb16_sb[:, 640:1152]]

            for h in range(NBH):
                base = LH * h  # halo-coord start of this half's x1/k/v span
                if h == 0:
                    x_sb = x_sb_h0
                else:
                    x_sb = {}
                    for p in range(2):
                        x_sb[p] = xpool.tile(
                            [128, 2, 2, KSPAN], F8, tag=f"x{p}", name=f"x{p}_{h}")
                    # split per chunk so projections start while the rest of
                    # the slice streams in (all h=0 pieces issued up top)
                    for p in range(2):
                        for j in range(2):
                            _x_dma(x_sb, p, j, 256, 1536, base)
                    for p in range(2):
                        for j in range(2):
                            _x_dma(x_sb, p, j, 1536, 2560, base)

                # ---- projections (fp8 DoubleRow, 3 hi/lo chains) ----
                q_sb, k_sb = [], []
                for cc in range(2):
                    q_sb.append(qkv.tile([128, LH], F16, tag=f"q{cc}", name=f"q{cc}_{h}"))
                    k_sb.append(qkv.tile([128, KSPAN], F16, tag=f"k{cc}", name=f"k{cc}_{h}"))

                CHAINS = ((0, 0), (1, 0), (0, 1))  # (w hi/lo, x hi/lo)

                def _proj_psum(kind, cc, ps, x0):
                    # accumulate W^T x into ps[128, 512] over K=512 via
                    # 2 DoubleRow pair-steps x 3 chains x 2 col-halves
                    csl = slice(128 * cc, 128 * (cc + 1))
                    for half_i in range(2):
                        n0 = x0 + 256 * half_i
                        first = True
                        for p in range(2):
                            for (wp, xp) in CHAINS:
                                nc.tensor.matmul(
                                    ps[:, 256 * half_i:256 * (half_i + 1)],
                                    wT_sb[kind][:, wp, p, :, csl],
                                    x_sb[p][:, xp, :, n0:n0 + 256],
                                    start=first, stop=(p == 1 and (wp, xp) == CHAINS[-1]),
                                    perf_mode=PM, skip_group_check=True,
                                )
                                first = False

                def k_group(mt):
                    for cc in range(2):
                        ps = pse.tile([128, 512], F32, tag="e",
                                      name=f"psk{h}{cc}{mt}")
                        _proj_psum("k", cc, ps, 512 * mt)
                        nc.vector.tensor_scalar_add(
                            k_sb[cc][:, 512 * mt:512 * (mt + 1)], ps, bk_sb[cc]
                        )

                def q_group(lt):
                    for cc in range(2):
                        ps = pse.tile([128, 512], F32, tag="e",
                                      name=f"psq{h}{cc}{lt}")
                        _proj_psum("q", cc, ps, HALF + 512 * lt)
                        nc.vector.tensor_scalar_add(
                            q_sb[cc][:, 512 * lt:512 * (lt + 1)], ps, bq_sb[cc]
                        )

                vT_sb = [None] * (KSPAN // 128)
                if h > 0:
                    # halo reuse: this half's m=0..3 v-chunks cover the same
                    # x1 columns as the previous half's m=16..19 — alias them
                    for mt in range(4):
                        vT_sb[mt] = prev_vT[16 + mt]

                def vT_group(mts):
                    for mt in mts:
                        ps = pso.tile([128, C], F32, tag="o", name=f"psv{h}{mt}")
                        first = True
                        for p in range(2):
                            for (wp, xp) in CHAINS:
                                nc.tensor.matmul(
                                    ps,
                                    x_sb[p][:, xp, :, 128 * mt:128 * (mt + 1)],
                                    wT_sb["v"][:, wp, p],
                                    start=first, stop=(p == 1 and (wp, xp) == CHAINS[-1]),
                                    perf_mode=PM, skip_group_check=True,
                                )
                                first = False
                        t = qkv.tile([128, C], F16, tag=f"v{mt}", name=f"vT{mt}_{h}")
                        # eviction with the (per-free-element) v bias folded in
                        nc.vector.tensor_tensor(t, ps, bvrep_sb, op=Alu.add)
                        vT_sb[mt] = t

                # ---- attention blocks (software-pipelined: block b's
                # colsum/AV/outproj are emitted after block b+1's energy+exp
                # so PE never waits on the ACT/Pool softmax chain) ----
                def emit_energy(h, b, k_sb=k_sb, q_sb=q_sb, vT_sb=vT_sb,
                                kh=(kh_prev if h > 0 else None)):
                    woff = 512 * b   # window start in k/vT coords
                    first_blk = (h == 0 and b == 0)
                    last_blk = (h == NBH - 1 and b == BPH - 1)
                    pt = {}
                    for r in AVORDER:
                        lo, w = INTERVALS[r]
                        ps_e = pse.tile([128, w], F32, tag="e", name=f"pse{h}{b}{r}")
                        for cc in range(2):
                            if kh is not None and b == 0 and r < 4:
                                klhs = kh[cc][:, 128 * r:128 * (r + 1)]
                            else:
                                klhs = k_sb[cc][:, woff + 128 * r:
                                                woff + 128 * (r + 1)]
                            nc.tensor.matmul(
                                ps_e, klhs,
                                q_sb[cc][:, 512 * b + lo: 512 * b + lo + w],
                                start=(cc == 0), stop=(cc == 1),
                                skip_group_check=True,
                            )
                        t = ptp.tile([128, w], F16, tag=f"pt{r}", name=f"pt{r}_{h}{b}")
                        nc.scalar.activation(t, ps_e, Act.Exp, scale=EXP_SCALE)
                        # zero outside the band: one affine compare per tile
                        # (lower bound bites for r<=3, upper for r>=4)
                        if r <= 3:
                            nc.gpsimd.affine_select(
                                out=t, in_=t, compare_op=Alu.is_ge, fill=0.0,
                                base=128 * r - lo, channel_multiplier=1,
                                pattern=[[-1, w]],
                            )
                        else:
                            # valid iff (128r+m')-l <= 511, recast as
                            # (511-128r+lo) - m' + j >= 0 (is_ge only)
                            nc.gpsimd.affine_select(
                                out=t, in_=t, compare_op=Alu.is_ge, fill=0.0,
                                base=(BL - 1) - 128 * r + lo,
                                channel_multiplier=-1,
                                pattern=[[1, w]],
                            )
                        if first_blk and r < 2:
                            nc.vector.tensor_scalar_mul(t, t, padf_sb[r])
                        elif last_blk and r >= 6:
                            nc.vector.tensor_scalar_mul(t, t, padl_sb[r - 6])
                        pt[r] = t
                    return (h, b, pt, vT_sb)

                def emit_tail(ctx):
                    h, b, pt, vT_l = ctx
                    ps_s = pss.tile([128, 512], F32, tag="s", name=f"pss{h}{b}")
                    for i, r in enumerate(AVORDER):
                        lo, w = INTERVALS[r]
                        nc.tensor.matmul(
                            ps_s[:, lo:lo + w], ones_sb, pt[r],
                            start=(i == 0), stop=(i == 7), skip_group_check=True,
                        )
                    recip = sbo.tile([128, 512], F32, tag="recip", name=f"rc{h}{b}")
                    nc.vector.reciprocal(recip, ps_s)

                    ps_av = []
                    for cc in range(2):
                        ps_av.append(psav.tile([128, 512], F32, tag=f"av{cc}",
                                               name=f"psav{h}{b}{cc}"))
                    for i, r in enumerate(AVORDER):
                        lo, w = INTERVALS[r]
                        for cc in range(2):
                            nc.tensor.matmul(
                                ps_av[cc][:, lo:lo + w],
                                vT_l[4 * b + r][:, 128 * cc:128 * (cc + 1)], pt[r],
                                start=(i == 0), stop=(i == 7), skip_group_check=True,
                            )
                    # normalization fused into the relu eviction: keeps the
                    # fp16 tile in range and shortens the output tail
                    relu_sb = []
                    for cc in range(2):
                        t = sbo.tile([128, 512], F16, tag=f"relu{cc}",
                                     name=f"relu{h}{b}{cc}")
                        nc.vector.scalar_tensor_tensor(
                            t, ps_av[cc], 0.0, recip, op0=Alu.max, op1=Alu.mult
                        )
                        relu_sb.append(t)

                    last_blk = (h == NBH - 1 and b == BPH - 1)
                    for oc in range(4):
                        ps_o = pso.tile([128, 512], F32, tag="o", name=f"pso{h}{b}{oc}")
                        for cc in range(2):
                            nc.tensor.matmul(
                                ps_o, woT_sb[cc][:, 128 * oc:128 * (oc + 1)],
                                relu_sb[cc], start=(cc == 0), stop=(cc == 1),
                            )
                        o_sb = sbo.tile([128, 512], F16, tag="osb", name=f"o{h}{b}{oc}")
                        # last block: alternate ACT/DVE so the four final
                        # evictions don't serialize on one engine
                        if last_blk and oc % 2:
                            nc.vector.scalar_tensor_tensor(
                                o_sb, ps_o, 0.0, recip, op0=Alu.bypass, op1=Alu.bypass
                            )
                        else:
                            nc.scalar.activation(o_sb, ps_o, Act.Copy)
                        nc.sync.dma_start(
                            out=out_d.ap()[128 * oc:128 * (oc + 1),
                                           LH * h + 512 * b: LH * h + 512 * (b + 1)],
                            in_=o_sb,
                        )

                pending = []
                for b in range(BPH):
                    if b == 0:
                        if h == 0:
                            k_group(0)
                        k_group(1)
                        q_group(0)
                        vT_group(range(0, 8) if h == 0 else range(4, 8))
                    else:
                        k_group(b + 1)
                        q_group(b)
                        vT_group(range(4 * b + 4, 4 * b + 8))
                    pending.append(emit_energy(h, b))
                    if len(pending) > 1:
                        emit_tail(pending.pop(0))
                # flush before the next half's projections overwrite q/k/vT
                for ctx in pending:
                    emit_tail(ctx)
                prev_vT = vT_sb
                if h == 0:
                    # stash the k halo overlap for the next half (SBUF->SBUF
                    # DMA, off-engine); half1's block 0 reads it directly
                    kh_prev = []
                    for cc in range(2):
                        tkh = qkv.tile([128, 512], F16, tag=f"kh{cc}",
                                       name=f"kh{cc}")
                        nc.sync.dma_start(out=tkh,
                                          in_=k_sb[cc][:, LH:LH + 512])
                        kh_prev.append(tkh)
    nc.compile()
    return nc


_NC_CACHE = {}


def _get_nc():
    if "nc" not in _NC_CACHE:
        _NC_CACHE["nc"] = _build_program()
    return _NC_CACHE["nc"]


def _f8():
    try:
        import ml_dtypes
        return ml_dtypes.float8_e4m3
    except ImportError:  # pragma: no cover
        import jax.numpy as jnp
        return jnp.float8_e4m3


def _split8(a):
    f8 = _f8()
    hi = np.asarray(a, np.float32).astype(f8)
    lo = (np.asarray(a, np.float32) - hi.astype(np.float32)).astype(f8)
    return hi, lo


def make_in_maps(x1, mask, Wq, bq, Wk, bk, Wv, bv, Wo, bo):
    x1 = np.asarray(x1, dtype=np.float32).reshape(CIN, L)

    def _pairs(w):
        # (C_out=256, C_in=512) -> [128, 2(hl), 2(p), 2(j), C] DoubleRow pair
        # layout with global c_in = 128*(2p+j) + c_in_within
        ws = np.asarray(w, np.float32) * WSCALE
        hi, lo = _split8(ws.T)          # (512, 256) each
        def lay(a):
            return a.reshape(2, 2, 128, C).transpose(2, 0, 1, 3)
        return np.ascontiguousarray(np.stack([lay(hi), lay(lo)], axis=1))

    wq8 = _pairs(Wq)
    wk8 = _pairs(Wk)
    wv8 = _pairs(Wv)
    woT = (np.asarray(Wo, np.float32).T / WSCALE).astype(np.float16)

    cb32 = np.zeros((128, 264), np.float32)
    cb32[:, 0:2] = (np.asarray(bq, np.float32) * WSCALE).reshape(2, 128).T
    cb32[:, 2:4] = (np.asarray(bk, np.float32) * WSCALE).reshape(2, 128).T
    # padf/padl cols 4:8 filled per core below
    cb32[:, 8:264] = np.broadcast_to(
        (np.asarray(bv, np.float32) * WSCALE).reshape(1, C), (128, C))

    cb16 = np.zeros((128, 1152), np.float16)
    cb16[:, 0:128] = 1.0
    cb16[:, 128:640] = woT[0:128]
    cb16[:, 640:1152] = woT[128:256]

    in_maps = []
    for c in range(NCORES):
        g0 = LC * c - HALF
        x1h = np.zeros((CIN, HALO), np.float32)
        s0, s1 = max(g0, 0), min(g0 + HALO, L)
        x1h[:, s0 - g0:s1 - g0] = x1[:, s0:s1]
        xh, xl = _split8(x1h)
        cb = cb32.copy()
        cb[:, 4:6] = 0.0 if c == 0 else 1.0
        cb[:, 6:8] = 0.0 if c == NCORES - 1 else 1.0
        m = {
            "xhl": np.ascontiguousarray(np.stack([xh, xl], axis=1)),
            "wq": wq8, "wk": wk8, "wv": wv8,
            "cb32": cb, "cb16": cb16,
        }
        in_maps.append(m)
    return in_maps


def postprocess(results, mask, bo):
    cols = np.concatenate(
        [np.asarray(results[c]["out"], np.float32) for c in range(NCORES)], axis=1)
    out = cols[None] + np.asarray(bo, np.float32)[None, :, None]
    return (out * np.asarray(mask, np.float32)).astype(np.float32)


def kernel(x1, x2, mask, Wq, bq, Wk, bk, Wv, bv, Wo, bo, **_unused):
    from concourse.bass_utils import run_bass_kernel_spmd

    nc = _get_nc()
    in_maps = make_in_maps(x1, mask, Wq, bq, Wk, bk, Wv, bv, Wo, bo)
    res = run_bass_kernel_spmd(nc, in_maps, core_ids=list(range(NCORES)))
    return postprocess(res.results, mask, bo)


# revision 16
# speedup vs baseline: 1.1631x; 1.0018x over previous
"""Sliding-window (banded) attention for nn_AttLayer on 8 Trainium2 NeuronCores.

Reference computation (per window-block n of 512 positions, 64 blocks over L=32768):
  q/k/v = 1x1-conv projections of x1 (512ch -> 256ch)
  energy[l, m] = (q_block[:, l] . k_window[:, m]) / 16   over a 1024-wide window
  attn = softmax(energy + log(band_mask + 1e-6)) * band_mask
  out  = relu(v_window @ attn^T) -> 1x1-conv (256 -> 512) + bias, masked

Sharding: 64 blocks split contiguously across 8 cores (8 blocks each). Each core
gets a zero-padded halo slice of x1 and computes its 4096 output columns.

Kernel strategy (per core, SPMD — all per-core variation is in the data):
  - Projections on PE in fp8e4 DoubleRow perf mode (0.5 cycles/row, two
    128-channel contraction tiles per instruction -> 4x MAC throughput).
    x1 and the projection weights are split host-side into e4m3 hi/lo pairs
    (W scaled by 64 to center the fp8 range; the 64^2 folds into the exp
    scale and 1/64 into Wo). Three accumulation chains (hi*hi + lo*hi +
    hi*lo) recover ~bf16-level accuracy at 0.75x the f32r cycle cost.
  - Everything downstream runs in fp16 (1 cycle/row like f32r but with no
    >=256 moving-width requirement, half the SBUF/DMA bytes of f32, and a
    10-bit mantissa -- ~10x less quantization error than bf16).
  - energy computed transposed: energyT[m, l] = k_chunk^T q (PE), over the
    EXACT per-chunk band intervals (2560 of 4096 window cols per block).
  - Band masking: affine_select on the otherwise-idle Pool/GPSIMD engine
    zeroes out-of-band exp values; sequence-edge padding handled with
    per-core 0/1 data vectors so the program stays SPMD.
  - exp on ScalarE (scale 2^-16 folds away the fp8 weight scaling),
    denominators via an all-ones fp16 matmul (column sums land replicated
    across partitions), reciprocal on VectorE.
  - AV on PE (fp16); softmax normalization + relu fused into the one
    PSUM->SBUF eviction (scalar_tensor_tensor max+mult) which also keeps
    the fp16 relu tile in range. Output projection on PE; its eviction is
    a plain ACT copy (psum -> fp16). Final bias/mask applied on host.
  - Blocks are software-pipelined (block b's colsum/AV/outproj emitted after
    block b+1's energy+exp) so PE never waits on the softmax chain, and the
    k/q/vT projection groups are interleaved INTO the block stream so the
    fresh-x DMA demand stays below the HBM wire rate.
  - Halo reuse between the two halves: half 1's first four vT tiles alias
    half 0's last four (identical x1 columns), and half 0's k overlap is
    stashed via an SBUF->SBUF DMA so half 1 skips its first k-group.
"""

import numpy as np

NCORES = 8
L = 32768
CIN = 512
C = 256
BL = 512
HALF = 256
LC = L // NCORES              # 4096 positions per core
HALO = LC + 2 * HALF          # 4608
NBH = 2                       # halves per core
LH = LC // NBH                # 2048 positions per half
KSPAN = LH + 2 * HALF         # 2560 k/v positions per half
BPH = 4                       # blocks per half
WSCALE = 64.0                 # host-side fp8 scaling of Wq/Wk/Wv (and biases)
EXP_SCALE = (1.0 / 16.0) / (WSCALE * WSCALE)   # softmax scale / W-scaling^2

# Per m-chunk r (8 chunks of the 1024-wide window): EXACT valid l-interval
# (lo, width) within the block's 512 queries (fp16 has no min-width penalty).
INTERVALS = [
    (0, 128), (0, 256), (0, 384), (0, 512),
    (0, 512), (128, 384), (256, 256), (384, 128),
]
# accumulation order: r=3 covers the full [0,512) so it goes first (start=True)
AVORDER = [3, 4, 2, 5, 1, 6, 0, 7]


def _build_program():
    import concourse.mybir as mybir
    from concourse import bacc
    from concourse.tile import TileContext

    F32 = mybir.dt.float32
    F16 = mybir.dt.float16
    F8 = mybir.dt.float8e4
    BF16 = mybir.dt.bfloat16
    Alu = mybir.AluOpType
    Act = mybir.ActivationFunctionType
    PM = mybir.MatmulPerfMode.DoubleRow

    nc = bacc.Bacc()

    # x hi/lo fp8 halo slice, hi and lo planes interleaved per channel so one
    # DMA fills both; weights in DoubleRow pair layout
    # [c_in_within_chunk(128), hi/lo, pair p, row j, c_out] with global input
    # channel 128*(2p+j) + c_in.
    xhl_d = nc.dram_tensor("xhl", [CIN, 2, HALO], F8, kind="ExternalInput")
    w_d = {}
    for kind in ("q", "k", "v"):
        w_d[kind] = nc.dram_tensor(
            f"w{kind}", [128, 2, 2, 2, C], F8, kind="ExternalInput")
    # f32 scalar blob: [bq0 bq1 bk0 bk1 padf0 padf1 padl0 padl1 | bvr(256)]
    cb32_d = nc.dram_tensor("cb32", [128, 264], F32, kind="ExternalInput")
    # f16 blob: [ones(128) | woT0(512) | woT1(512)]
    cb16_d = nc.dram_tensor("cb16", [128, 1152], F16, kind="ExternalInput")
    # c-major output: full channel g = 128*oc + c lives at [c, oc, :] — lets
    # one DMA carry several oc chunks (host re-interleaves)
    out_d = nc.dram_tensor("out", [128, 4, LC], F16, kind="ExternalOutput")

    with TileContext(nc) as tc:
        with (
            tc.tile_pool(name="consts", bufs=1) as consts,
            tc.tile_pool(name="xpool", bufs=2) as xpool,
            tc.tile_pool(name="qkv", bufs=1) as qkv,
            tc.tile_pool(name="ptp", bufs=2) as ptp,
            tc.tile_pool(name="sbo", bufs=4) as sbo,
            tc.tile_pool(name="pse", bufs=3, space="PSUM") as pse,
            tc.tile_pool(name="pss", bufs=1, space="PSUM") as pss,
            tc.tile_pool(name="psav", bufs=1, space="PSUM") as psav,
            tc.tile_pool(name="pso", bufs=2, space="PSUM") as pso,
        ):
            # warm the ACT exp table while DMAs stream in
            warm_sb = consts.tile([1, 8], F32)
            nc.vector.memset(warm_sb, 0.0)
            nc.scalar.activation(warm_sb, warm_sb, Act.Exp)

            # warm the PE clock gate (HAM) during the initial DMA wait:
            # dummy bf16 matmuls on memset data keep the array busy so the
            # first real projections run at the full 2.4 GHz
            warm_a = consts.tile([128, 128], BF16, name="warm_a")
            nc.vector.memset(warm_a, 1.0)
            warm_b = consts.tile([128, 512], BF16, name="warm_b")
            nc.vector.memset(warm_b, 1.0)
            for wi in range(8):
                warm_ps = pse.tile([128, 512], F32, tag="e", name=f"wps{wi}")
                nc.tensor.matmul(warm_ps, warm_a, warm_b, start=True, stop=True)

            # critical-path-first DMA order: the first PE work is the h=0
            # k-projection of columns [0:512), needing wk and x chunk 0
            wT_sb = {}
            t = consts.tile([128, 2, 2, 2, C], F8, name="wk")
            nc.sync.dma_start(out=t, in_=w_d["k"].ap())
            wT_sb["k"] = t
            # x pair tiles: [128, hi/lo, row j, pos] per pair p
            x_sb_h0 = {}
            for p in range(2):
                x_sb_h0[p] = xpool.tile(
                    [128, 2, 2, KSPAN], F8, tag=f"x{p}", name=f"x{p}_0")

            def _x_dma(x_sb, p, j, a, b, base):
                g0 = 128 * (2 * p + j)
                nc.sync.dma_start(
                    out=x_sb[p][:, :, j, a:b],
                    in_=xhl_d.ap()[g0:g0 + 128, :, base + a:base + b],
                )

            def _x0_piece(ct, n=1):
                for p in range(2):
                    for j in range(2):
                        _x_dma(x_sb_h0, p, j, 512 * ct, 512 * (ct + n), 0)

            _x0_piece(0)
            cb32_sb = consts.tile([128, 264], F32, name="cb32")
            nc.sync.dma_start(out=cb32_sb, in_=cb32_d.ap())
            bq_sb = [cb32_sb[:, 0:1], cb32_sb[:, 1:2]]
            bk_sb = [cb32_sb[:, 2:3], cb32_sb[:, 3:4]]
            padf_sb = [cb32_sb[:, 4:5], cb32_sb[:, 5:6]]
            padl_sb = [cb32_sb[:, 6:7], cb32_sb[:, 7:8]]
            bvrep_sb = cb32_sb[:, 8:264]
            _x0_piece(1, 2)
            t = consts.tile([128, 2, 2, 2, C], F8, name="wq")
            nc.sync.dma_start(out=t, in_=w_d["q"].ap())
            wT_sb["q"] = t
            t = consts.tile([128, 2, 2, 2, C], F8, name="wv")
            nc.sync.dma_start(out=t, in_=w_d["v"].ap())
            wT_sb["v"] = t
            _x0_piece(3, 2)

            cb16_sb = consts.tile([128, 1152], F16, name="cb16")
            nc.sync.dma_start(out=cb16_sb, in_=cb16_d.ap())
            ones_sb = cb16_sb[:, 0:128]
            woT_sb = [cb16_sb[:, 128:640], cb16_sb[:, 640:1152]]

            for h in range(NBH):
                base = LH * h  # halo-coord start of this half's x1/k/v span
                if h == 0:
                    x_sb = x_sb_h0
                else:
                    x_sb = {}
                    for p in range(2):
                        x_sb[p] = xpool.tile(
                            [128, 2, 2, KSPAN], F8, tag=f"x{p}", name=f"x{p}_{h}")
                    # split per chunk so projections start while the rest of
                    # the slice streams in (all h=0 pieces issued up top)
                    for p in range(2):
                        for j in range(2):
                            _x_dma(x_sb, p, j, 256, 1536, base)
                    for p in range(2):
                        for j in range(2):
                            _x_dma(x_sb, p, j, 1536, 2560, base)

                # ---- projections (fp8 DoubleRow, 3 hi/lo chains) ----
                q_sb, k_sb = [], []
                for cc in range(2):
                    q_sb.append(qkv.tile([128, LH], F16, tag=f"q{cc}", name=f"q{cc}_{h}"))
                    k_sb.append(qkv.tile([128, KSPAN], F16, tag=f"k{cc}", name=f"k{cc}_{h}"))

                CHAINS = ((0, 0), (1, 0), (0, 1))  # (w hi/lo, x hi/lo)

                def _proj_psum(kind, cc, ps, x0):
                    # accumulate W^T x into ps[128, 512] over K=512 via
                    # 2 DoubleRow pair-steps x 3 chains x 2 col-halves
                    csl = slice(128 * cc, 128 * (cc + 1))
                    for half_i in range(2):
                        n0 = x0 + 256 * half_i
                        first = True
                        for p in range(2):
                            for (wp, xp) in CHAINS:
                                nc.tensor.matmul(
                                    ps[:, 256 * half_i:256 * (half_i + 1)],
                                    wT_sb[kind][:, wp, p, :, csl],
                                    x_sb[p][:, xp, :, n0:n0 + 256],
                                    start=first, stop=(p == 1 and (wp, xp) == CHAINS[-1]),
                                    perf_mode=PM, skip_group_check=True,
                                )
                                first = False

                def k_group(mt):
                    for cc in range(2):
                        ps = pse.tile([128, 512], F32, tag="e",
                                      name=f"psk{h}{cc}{mt}")
                        _proj_psum("k", cc, ps, 512 * mt)
                        nc.vector.tensor_scalar_add(
                            k_sb[cc][:, 512 * mt:512 * (mt + 1)], ps, bk_sb[cc]
                        )

                def q_group(lt):
                    for cc in range(2):
                        ps = pse.tile([128, 512], F32, tag="e",
                                      name=f"psq{h}{cc}{lt}")
                        _proj_psum("q", cc, ps, HALF + 512 * lt)
                        nc.vector.tensor_scalar_add(
                            q_sb[cc][:, 512 * lt:512 * (lt + 1)], ps, bq_sb[cc]
                        )

                vT_sb = [None] * (KSPAN // 128)
                if h > 0:
                    # halo reuse: this half's m=0..3 v-chunks cover the same
                    # x1 columns as the previous half's m=16..19 — alias them
                    for mt in range(4):
                        vT_sb[mt] = prev_vT[16 + mt]

                def vT_group(mts):
                    for mt in mts:
                        ps = pso.tile([128, C], F32, tag="o", name=f"psv{h}{mt}")
                        first = True
                        for p in range(2):
                            for (wp, xp) in CHAINS:
                                nc.tensor.matmul(
                                    ps,
                                    x_sb[p][:, xp, :, 128 * mt:128 * (mt + 1)],
                                    wT_sb["v"][:, wp, p],
                                    start=first, stop=(p == 1 and (wp, xp) == CHAINS[-1]),
                                    perf_mode=PM, skip_group_check=True,
                                )
                                first = False
                        t = qkv.tile([128, C], F16, tag=f"v{mt}", name=f"vT{mt}_{h}")
                        # eviction with the (per-free-element) v bias folded in
                        nc.vector.tensor_tensor(t, ps, bvrep_sb, op=Alu.add)
                        vT_sb[mt] = t

                # ---- attention blocks (software-pipelined: block b's
                # colsum/AV/outproj are emitted after block b+1's energy+exp
                # so PE never waits on the ACT/Pool softmax chain) ----
                def emit_energy(h, b, k_sb=k_sb, q_sb=q_sb, vT_sb=vT_sb,
                                kh=(kh_prev if h > 0 else None)):
                    woff = 512 * b   # window start in k/vT coords
                    first_blk = (h == 0 and b == 0)
                    last_blk = (h == NBH - 1 and b == BPH - 1)
                    pt = {}
                    for r in AVORDER:
                        lo, w = INTERVALS[r]
                        ps_e = pse.tile([128, w], F32, tag="e", name=f"pse{h}{b}{r}")
                        for cc in range(2):
                            if kh is not None and b == 0 and r < 4:
                                klhs = kh[cc][:, 128 * r:128 * (r + 1)]
                            else:
                                klhs = k_sb[cc][:, woff + 128 * r:
                                                woff + 128 * (r + 1)]
                            nc.tensor.matmul(
                                ps_e, klhs,
                                q_sb[cc][:, 512 * b + lo: 512 * b + lo + w],
                                start=(cc == 0), stop=(cc == 1),
                                skip_group_check=True,
                            )
                        t = ptp.tile([128, w], F16, tag=f"pt{r}", name=f"pt{r}_{h}{b}")
                        nc.scalar.activation(t, ps_e, Act.Exp, scale=EXP_SCALE)
                        # zero outside the band: one affine compare per tile
                        # (lower bound bites for r<=3, upper for r>=4)
                        if r <= 3:
                            nc.gpsimd.affine_select(
                                out=t, in_=t, compare_op=Alu.is_ge, fill=0.0,
                                base=128 * r - lo, channel_multiplier=1,
                                pattern=[[-1, w]],
                            )
                        else:
                            # valid iff (128r+m')-l <= 511, recast as
                            # (511-128r+lo) - m' + j >= 0 (is_ge only)
                            nc.gpsimd.affine_select(
                                out=t, in_=t, compare_op=Alu.is_ge, fill=0.0,
                                base=(BL - 1) - 128 * r + lo,
                                channel_multiplier=-1,
                                pattern=[[1, w]],
                            )
                        if first_blk and r < 2:
                            nc.vector.tensor_scalar_mul(t, t, padf_sb[r])
                        elif last_blk and r >= 6:
                            nc.vector.tensor_scalar_mul(t, t, padl_sb[r - 6])
                        pt[r] = t
                    return (h, b, pt, vT_sb)

                def emit_tail(ctx):
                    h, b, pt, vT_l = ctx
                    ps_s = pss.tile([128, 512], F32, tag="s", name=f"pss{h}{b}")
                    for i, r in enumerate(AVORDER):
                        lo, w = INTERVALS[r]
                        nc.tensor.matmul(
                            ps_s[:, lo:lo + w], ones_sb, pt[r],
                            start=(i == 0), stop=(i == 7), skip_group_check=True,
                        )
                    recip = sbo.tile([128, 512], F32, tag="recip", name=f"rc{h}{b}")
                    nc.vector.reciprocal(recip, ps_s)

                    ps_av = []
                    for cc in range(2):
                        ps_av.append(psav.tile([128, 512], F32, tag=f"av{cc}",
                                               name=f"psav{h}{b}{cc}"))
                    for i, r in enumerate(AVORDER):
                        lo, w = INTERVALS[r]
                        for cc in range(2):
                            nc.tensor.matmul(
                                ps_av[cc][:, lo:lo + w],
                                vT_l[4 * b + r][:, 128 * cc:128 * (cc + 1)], pt[r],
                                start=(i == 0), stop=(i == 7), skip_group_check=True,
                            )
                    # normalization fused into the relu eviction: keeps the
                    # fp16 tile in range and shortens the output tail
                    relu_sb = []
                    for cc in range(2):
                        t = sbo.tile([128, 512], F16, tag=f"relu{cc}",
                                     name=f"relu{h}{b}{cc}")
                        nc.vector.scalar_tensor_tensor(
                            t, ps_av[cc], 0.0, recip, op0=Alu.max, op1=Alu.mult
                        )
                        relu_sb.append(t)

                    last_blk = (h == NBH - 1 and b == BPH - 1)
                    o_sb = sbo.tile([128, 4, 512], F16, tag="osb", name=f"o{h}{b}")
                    for oc in range(4):
                        ps_o = pso.tile([128, 512], F32, tag="o", name=f"pso{h}{b}{oc}")
                        for cc in range(2):
                            nc.tensor.matmul(
                                ps_o, woT_sb[cc][:, 128 * oc:128 * (oc + 1)],
                                relu_sb[cc], start=(cc == 0), stop=(cc == 1),
                            )
                        # last block: alternate ACT/DVE so the four final
                        # evictions don't serialize on one engine
                        if last_blk and oc % 2:
                            nc.vector.scalar_tensor_tensor(
                                o_sb[:, oc], ps_o, 0.0, recip,
                                op0=Alu.bypass, op1=Alu.bypass,
                            )
                        else:
                            nc.scalar.activation(o_sb[:, oc], ps_o, Act.Copy)
                        if oc % 2:
                            nc.sync.dma_start(
                                out=out_d.ap()[:, oc - 1:oc + 1,
                                               LH * h + 512 * b: LH * h + 512 * (b + 1)],
                                in_=o_sb[:, oc - 1:oc + 1],
                            )

                pending = []
                for b in range(BPH):
                    if b == 0:
                        if h == 0:
                            k_group(0)
                        k_group(1)
                        q_group(0)
                        vT_group(range(0, 8) if h == 0 else range(4, 8))
                    else:
                        k_group(b + 1)
                        q_group(b)
                        vT_group(range(4 * b + 4, 4 * b + 8))
                    pending.append(emit_energy(h, b))
                    if len(pending) > 1:
                        emit_tail(pending.pop(0))
                # flush before the next half's projections overwrite q/k/vT
                for ctx in pending:
                    emit_tail(ctx)
                prev_vT = vT_sb
                if h == 0:
                    # stash the k halo overlap for the next half (SBUF->SBUF
                    # DMA, off-engine); half1's block 0 reads it directly
                    kh_prev = []
                    for cc in range(2):
                        tkh = qkv.tile([128, 512], F16, tag=f"kh{cc}",
                                       name=f"kh{cc}")
                        nc.sync.dma_start(out=tkh,
                                          in_=k_sb[cc][:, LH:LH + 512])
                        kh_prev.append(tkh)
    nc.compile()
    return nc


_NC_CACHE = {}


def _get_nc():
    if "nc" not in _NC_CACHE:
        _NC_CACHE["nc"] = _build_program()
    return _NC_CACHE["nc"]


def _f8():
    try:
        import ml_dtypes
        return ml_dtypes.float8_e4m3
    except ImportError:  # pragma: no cover
        import jax.numpy as jnp
        return jnp.float8_e4m3


def _split8(a):
    f8 = _f8()
    hi = np.asarray(a, np.float32).astype(f8)
    lo = (np.asarray(a, np.float32) - hi.astype(np.float32)).astype(f8)
    return hi, lo


def make_in_maps(x1, mask, Wq, bq, Wk, bk, Wv, bv, Wo, bo):
    x1 = np.asarray(x1, dtype=np.float32).reshape(CIN, L)

    def _pairs(w):
        # (C_out=256, C_in=512) -> [128, 2(hl), 2(p), 2(j), C] DoubleRow pair
        # layout with global c_in = 128*(2p+j) + c_in_within
        ws = np.asarray(w, np.float32) * WSCALE
        hi, lo = _split8(ws.T)          # (512, 256) each
        def lay(a):
            return a.reshape(2, 2, 128, C).transpose(2, 0, 1, 3)
        return np.ascontiguousarray(np.stack([lay(hi), lay(lo)], axis=1))

    wq8 = _pairs(Wq)
    wk8 = _pairs(Wk)
    wv8 = _pairs(Wv)
    woT = (np.asarray(Wo, np.float32).T / WSCALE).astype(np.float16)

    cb32 = np.zeros((128, 264), np.float32)
    cb32[:, 0:2] = (np.asarray(bq, np.float32) * WSCALE).reshape(2, 128).T
    cb32[:, 2:4] = (np.asarray(bk, np.float32) * WSCALE).reshape(2, 128).T
    # padf/padl cols 4:8 filled per core below
    cb32[:, 8:264] = np.broadcast_to(
        (np.asarray(bv, np.float32) * WSCALE).reshape(1, C), (128, C))

    cb16 = np.zeros((128, 1152), np.float16)
    cb16[:, 0:128] = 1.0
    cb16[:, 128:640] = woT[0:128]
    cb16[:, 640:1152] = woT[128:256]

    in_maps = []
    for c in range(NCORES):
        g0 = LC * c - HALF
        x1h = np.zeros((CIN, HALO), np.float32)
        s0, s1 = max(g0, 0), min(g0 + HALO, L)
        x1h[:, s0 - g0:s1 - g0] = x1[:, s0:s1]
        xh, xl = _split8(x1h)
        cb = cb32.copy()
        cb[:, 4:6] = 0.0 if c == 0 else 1.0
        cb[:, 6:8] = 0.0 if c == NCORES - 1 else 1.0
        m = {
            "xhl": np.ascontiguousarray(np.stack([xh, xl], axis=1)),
            "wq": wq8, "wk": wk8, "wv": wv8,
            "cb32": cb, "cb16": cb16,
        }
        in_maps.append(m)
    return in_maps


def postprocess(results, mask, bo):
    # per-core out is [128, 4, LC] c-major; channel g = 128*oc + c
    cols = np.concatenate(
        [np.asarray(results[c]["out"], np.float32).transpose(1, 0, 2)
         .reshape(CIN, LC) for c in range(NCORES)], axis=1)
    out = cols[None] + np.asarray(bo, np.float32)[None, :, None]
    return (out * np.asarray(mask, np.float32)).astype(np.float32)


def kernel(x1, x2, mask, Wq, bq, Wk, bk, Wv, bv, Wo, bo, **_unused):
    from concourse.bass_utils import run_bass_kernel_spmd

    nc = _get_nc()
    in_maps = make_in_maps(x1, mask, Wq, bq, Wk, bk, Wv, bv, Wo, bo)
    res = run_bass_kernel_spmd(nc, in_maps, core_ids=list(range(NCORES)))
    return postprocess(res.results, mask, bo)


# revision 23
# speedup vs baseline: 1.1793x; 1.0140x over previous
"""Sliding-window (banded) attention for nn_AttLayer on 8 Trainium2 NeuronCores.

Reference computation (per window-block n of 512 positions, 64 blocks over L=32768):
  q/k/v = 1x1-conv projections of x1 (512ch -> 256ch)
  energy[l, m] = (q_block[:, l] . k_window[:, m]) / 16   over a 1024-wide window
  attn = softmax(energy + log(band_mask + 1e-6)) * band_mask
  out  = relu(v_window @ attn^T) -> 1x1-conv (256 -> 512) + bias, masked

Sharding: 64 blocks split contiguously across 8 cores (8 blocks each). Each core
gets a zero-padded halo slice of x1 and computes its 4096 output columns.

Kernel strategy (per core, SPMD — all per-core variation is in the data):
  - Projections on PE in fp8e4 DoubleRow perf mode (0.5 cycles/row, two
    128-channel contraction tiles per instruction -> 4x MAC throughput).
    x1 and the projection weights are split host-side into e4m3 hi/lo pairs
    (W scaled by 64 to center the fp8 range; the 64^2 folds into the exp
    scale and 1/64 into Wo). Three accumulation chains (hi*hi + lo*hi +
    hi*lo) recover ~bf16-level accuracy at 0.75x the f32r cycle cost.
  - Everything downstream runs in fp16 (1 cycle/row like f32r but with no
    >=256 moving-width requirement, half the SBUF/DMA bytes of f32, and a
    10-bit mantissa -- ~10x less quantization error than bf16).
  - energy computed transposed: energyT[m, l] = k_chunk^T q (PE), over the
    EXACT per-chunk band intervals (2560 of 4096 window cols per block).
  - Band masking: affine_select on the otherwise-idle Pool/GPSIMD engine
    zeroes out-of-band exp values; sequence-edge padding handled with
    per-core 0/1 data vectors so the program stays SPMD.
  - exp on ScalarE (scale 2^-16 folds away the fp8 weight scaling),
    denominators via an all-ones fp16 matmul (column sums land replicated
    across partitions), reciprocal on VectorE.
  - AV on PE (fp16); softmax normalization + relu fused into the one
    PSUM->SBUF eviction (scalar_tensor_tensor max+mult) which also keeps
    the fp16 relu tile in range. Output projection on PE; its eviction is
    a plain ACT copy (psum -> fp16). Final bias/mask applied on host.
  - Blocks are software-pipelined (block b's colsum/AV/outproj emitted after
    block b+1's energy+exp) so PE never waits on the softmax chain, and the
    k/q/vT projection groups are interleaved INTO the block stream so the
    fresh-x DMA demand stays below the HBM wire rate.
  - Halo reuse between the two halves: half 1's first four vT tiles alias
    half 0's last four (identical x1 columns), and half 0's k overlap is
    stashed via an SBUF->SBUF DMA so half 1 skips its first k-group.
"""

import numpy as np

NCORES = 8
L = 32768
CIN = 512
C = 256
BL = 512
HALF = 256
LC = L // NCORES              # 4096 positions per core
HALO = LC + 2 * HALF          # 4608
NBH = 2                       # halves per core
LH = LC // NBH                # 2048 positions per half
KSPAN = LH + 2 * HALF         # 2560 k/v positions per half
BPH = 4                       # blocks per half
WSCALE = 64.0                 # host-side fp8 scaling of Wq/Wk/Wv (and biases)
EXP_SCALE = (1.0 / 16.0) / (WSCALE * WSCALE)   # softmax scale / W-scaling^2

# Per m-chunk r (8 chunks of the 1024-wide window): EXACT valid l-interval
# (lo, width) within the block's 512 queries (fp16 has no min-width penalty).
INTERVALS = [
    (0, 128), (0, 256), (0, 384), (0, 512),
    (0, 512), (128, 384), (256, 256), (384, 128),
]
# accumulation order: r=3 covers the full [0,512) so it goes first (start=True)
AVORDER = [3, 4, 2, 5, 1, 6, 0, 7]


def _build_program():
    import concourse.mybir as mybir
    from concourse import bacc
    from concourse.tile import TileContext

    F32 = mybir.dt.float32
    F16 = mybir.dt.float16
    F8 = mybir.dt.float8e4
    BF16 = mybir.dt.bfloat16
    Alu = mybir.AluOpType
    Act = mybir.ActivationFunctionType
    PM = mybir.MatmulPerfMode.DoubleRow

    nc = bacc.Bacc()

    # x hi/lo fp8 halo slice in SBUF-tile order [pair p, c_within(128), hi/lo,
    # row j, pos] so ONE DMA per pair fills hi+lo and both j rows; weights in
    # DoubleRow pair layout [c_in_within_chunk(128), hi/lo, pair p, row j,
    # c_out] with global input channel 128*(2p+j) + c_in.
    xhl_d = nc.dram_tensor("xhl", [2, 128, 2, 2, HALO], F8, kind="ExternalInput")
    w_d = {}
    for kind in ("q", "k", "v"):
        w_d[kind] = nc.dram_tensor(
            f"w{kind}", [128, 2, 2, 2, C], F8, kind="ExternalInput")
    # f32 scalar blob: [bq0 bq1 bk0 bk1 padf0 padf1 padl0 padl1 | bvr(256)]
    cb32_d = nc.dram_tensor("cb32", [128, 264], F32, kind="ExternalInput")
    # f16 blob: [ones(128) | woT0(512) | woT1(512)]
    cb16_d = nc.dram_tensor("cb16", [128, 1152], F16, kind="ExternalInput")
    # c-major output: full channel g = 128*oc + c lives at [c, oc, :] — lets
    # one DMA carry several oc chunks (host re-interleaves)
    out_d = nc.dram_tensor("out", [128, 4, LC], F16, kind="ExternalOutput")

    with TileContext(nc) as tc:
        with (
            tc.tile_pool(name="consts", bufs=1) as consts,
            tc.tile_pool(name="xpool", bufs=2) as xpool,
            tc.tile_pool(name="qkv", bufs=1) as qkv,
            tc.tile_pool(name="ptp", bufs=2) as ptp,
            tc.tile_pool(name="sbo", bufs=4) as sbo,
            tc.tile_pool(name="pse", bufs=3, space="PSUM") as pse,
            tc.tile_pool(name="pss", bufs=1, space="PSUM") as pss,
            tc.tile_pool(name="psav", bufs=1, space="PSUM") as psav,
            tc.tile_pool(name="pso", bufs=2, space="PSUM") as pso,
        ):
            # warm the ACT exp table while DMAs stream in
            warm_sb = consts.tile([1, 8], F32)
            nc.vector.memset(warm_sb, 0.0)
            nc.scalar.activation(warm_sb, warm_sb, Act.Exp)

            # warm the PE clock gate (HAM) during the initial DMA wait:
            # dummy bf16 matmuls on memset data keep the array busy so the
            # first real projections run at the full 2.4 GHz
            warm_a = consts.tile([128, 128], BF16, name="warm_a")
            nc.vector.memset(warm_a, 1.0)
            warm_b = consts.tile([128, 512], BF16, name="warm_b")
            nc.vector.memset(warm_b, 1.0)
            for wi in range(8):
                warm_ps = pse.tile([128, 512], F32, tag="e", name=f"wps{wi}")
                nc.tensor.matmul(warm_ps, warm_a, warm_b, start=True, stop=True)

            # critical-path-first DMA order: the first PE work is the h=0
            # k-projection of columns [0:512), needing wk and x chunk 0
            wT_sb = {}
            t = consts.tile([128, 2, 2, 2, C], F8, name="wk")
            nc.sync.dma_start(out=t, in_=w_d["k"].ap())
            wT_sb["k"] = t
            # x pair tiles: [128, hi/lo, row j, pos] per pair p
            x_sb_h0 = {}
            for p in range(2):
                x_sb_h0[p] = xpool.tile(
                    [128, 2, 2, KSPAN], F8, tag=f"x{p}", name=f"x{p}_0")

            def _x_dma(x_sb, p, a, b, base):
                nc.sync.dma_start(
                    out=x_sb[p][:, :, :, a:b],
                    in_=xhl_d.ap()[p][:, :, :, base + a:base + b],
                )

            def _x0_piece(ct, n=1):
                for p in range(2):
                    _x_dma(x_sb_h0, p, 512 * ct, 512 * (ct + n), 0)

            _x0_piece(0)
            cb32_sb = consts.tile([128, 264], F32, name="cb32")
            nc.sync.dma_start(out=cb32_sb, in_=cb32_d.ap())
            bq_sb = [cb32_sb[:, 0:1], cb32_sb[:, 1:2]]
            bk_sb = [cb32_sb[:, 2:3], cb32_sb[:, 3:4]]
            padf_sb = [cb32_sb[:, 4:5], cb32_sb[:, 5:6]]
            padl_sb = [cb32_sb[:, 6:7], cb32_sb[:, 7:8]]
            bvrep_sb = cb32_sb[:, 8:264]
            _x0_piece(1, 2)
            t = consts.tile([128, 2, 2, 2, C], F8, name="wq")
            nc.sync.dma_start(out=t, in_=w_d["q"].ap())
            wT_sb["q"] = t
            t = consts.tile([128, 2, 2, 2, C], F8, name="wv")
            nc.sync.dma_start(out=t, in_=w_d["v"].ap())
            wT_sb["v"] = t
            _x0_piece(3, 2)

            cb16_sb = consts.tile([128, 1152], F16, name="cb16")
            nc.sync.dma_start(out=cb16_sb, in_=cb16_d.ap())
            ones_sb = cb16_sb[:, 0:128]
            woT_sb = [cb16_sb[:, 128:640], cb16_sb[:, 640:1152]]

            for h in range(NBH):
                base = LH * h  # halo-coord start of this half's x1/k/v span
                if h == 0:
                    x_sb = x_sb_h0
                else:
                    x_sb = {}
                    for p in range(2):
                        x_sb[p] = xpool.tile(
                            [128, 2, 2, KSPAN], F8, tag=f"x{p}", name=f"x{p}_{h}")
                    # split per chunk so projections start while the rest of
                    # the slice streams in (all h=0 pieces issued up top)
                    for p in range(2):
                        _x_dma(x_sb, p, 256, 1536, base)
                    for p in range(2):
                        _x_dma(x_sb, p, 1536, 2560, base)

                # ---- projections (fp8 DoubleRow, 3 hi/lo chains) ----
                q_sb, k_sb = [], []
                for cc in range(2):
                    q_sb.append(qkv.tile([128, LH], F16, tag=f"q{cc}", name=f"q{cc}_{h}"))
                    k_sb.append(qkv.tile([128, KSPAN], F16, tag=f"k{cc}", name=f"k{cc}_{h}"))

                CHAINS = ((0, 0), (1, 0), (0, 1))  # (w hi/lo, x hi/lo)

                def _proj_psum(kind, cc, ps, x0):
                    # accumulate W^T x into ps[128, 512] over K=512 via
                    # 2 DoubleRow pair-steps x 3 chains x 2 col-halves
                    csl = slice(128 * cc, 128 * (cc + 1))
                    for half_i in range(2):
                        n0 = x0 + 256 * half_i
                        first = True
                        for p in range(2):
                            for (wp, xp) in CHAINS:
                                nc.tensor.matmul(
                                    ps[:, 256 * half_i:256 * (half_i + 1)],
                                    wT_sb[kind][:, wp, p, :, csl],
                                    x_sb[p][:, xp, :, n0:n0 + 256],
                                    start=first, stop=(p == 1 and (wp, xp) == CHAINS[-1]),
                                    perf_mode=PM, skip_group_check=True,
                                )
                                first = False

                def k_group(mt):
                    for cc in range(2):
                        ps = pse.tile([128, 512], F32, tag="e",
                                      name=f"psk{h}{cc}{mt}")
                        _proj_psum("k", cc, ps, 512 * mt)
                        nc.vector.tensor_scalar_add(
                            k_sb[cc][:, 512 * mt:512 * (mt + 1)], ps, bk_sb[cc]
                        )

                def q_group(lt):
                    for cc in range(2):
                        ps = pse.tile([128, 512], F32, tag="e",
                                      name=f"psq{h}{cc}{lt}")
                        _proj_psum("q", cc, ps, HALF + 512 * lt)
                        nc.vector.tensor_scalar_add(
                            q_sb[cc][:, 512 * lt:512 * (lt + 1)], ps, bq_sb[cc]
                        )

                vT_sb = [None] * (KSPAN // 128)
                if h > 0:
                    # halo reuse: this half's m=0..3 v-chunks cover the same
                    # x1 columns as the previous half's m=16..19 — alias them
                    for mt in range(4):
                        vT_sb[mt] = prev_vT[16 + mt]

                def vT_group(mts):
                    for mt in mts:
                        ps = pso.tile([128, C], F32, tag="o", name=f"psv{h}{mt}")
                        first = True
                        for p in range(2):
                            for (wp, xp) in CHAINS:
                                nc.tensor.matmul(
                                    ps,
                                    x_sb[p][:, xp, :, 128 * mt:128 * (mt + 1)],
                                    wT_sb["v"][:, wp, p],
                                    start=first, stop=(p == 1 and (wp, xp) == CHAINS[-1]),
                                    perf_mode=PM, skip_group_check=True,
                                )
                                first = False
                        t = qkv.tile([128, C], F16, tag=f"v{mt}", name=f"vT{mt}_{h}")
                        # eviction with the (per-free-element) v bias folded in
                        nc.vector.tensor_tensor(t, ps, bvrep_sb, op=Alu.add)
                        vT_sb[mt] = t

                # ---- attention blocks (software-pipelined: block b's
                # colsum/AV/outproj are emitted after block b+1's energy+exp
                # so PE never waits on the ACT/Pool softmax chain) ----
                def emit_energy(h, b, k_sb=k_sb, q_sb=q_sb, vT_sb=vT_sb,
                                kh=(kh_prev if h > 0 else None)):
                    woff = 512 * b   # window start in k/vT coords
                    first_blk = (h == 0 and b == 0)
                    last_blk = (h == NBH - 1 and b == BPH - 1)
                    pt = {}
                    for r in AVORDER:
                        lo, w = INTERVALS[r]
                        ps_e = pse.tile([128, w], F32, tag="e", name=f"pse{h}{b}{r}")
                        for cc in range(2):
                            if kh is not None and b == 0 and r < 4:
                                klhs = kh[cc][:, 128 * r:128 * (r + 1)]
                            else:
                                klhs = k_sb[cc][:, woff + 128 * r:
                                                woff + 128 * (r + 1)]
                            nc.tensor.matmul(
                                ps_e, klhs,
                                q_sb[cc][:, 512 * b + lo: 512 * b + lo + w],
                                start=(cc == 0), stop=(cc == 1),
                                skip_group_check=True,
                            )
                        t = ptp.tile([128, w], F16, tag=f"pt{r}", name=f"pt{r}_{h}{b}")
                        nc.scalar.activation(t, ps_e, Act.Exp, scale=EXP_SCALE)
                        # zero outside the band: one affine compare per tile
                        # (lower bound bites for r<=3, upper for r>=4)
                        if r <= 3:
                            nc.gpsimd.affine_select(
                                out=t, in_=t, compare_op=Alu.is_ge, fill=0.0,
                                base=128 * r - lo, channel_multiplier=1,
                                pattern=[[-1, w]],
                            )
                        else:
                            # valid iff (128r+m')-l <= 511, recast as
                            # (511-128r+lo) - m' + j >= 0 (is_ge only)
                            nc.gpsimd.affine_select(
                                out=t, in_=t, compare_op=Alu.is_ge, fill=0.0,
                                base=(BL - 1) - 128 * r + lo,
                                channel_multiplier=-1,
                                pattern=[[1, w]],
                            )
                        if first_blk and r < 2:
                            nc.vector.tensor_scalar_mul(t, t, padf_sb[r])
                        elif last_blk and r >= 6:
                            nc.vector.tensor_scalar_mul(t, t, padl_sb[r - 6])
                        pt[r] = t
                    return (h, b, pt, vT_sb)

                def emit_tail(ctx):
                    h, b, pt, vT_l = ctx
                    last_blk = (h == NBH - 1 and b == BPH - 1)
                    ps_s = pss.tile([128, 512], F32, tag="s", name=f"pss{h}{b}")
                    for i, r in enumerate(AVORDER):
                        lo, w = INTERVALS[r]
                        nc.tensor.matmul(
                            ps_s[:, lo:lo + w], ones_sb, pt[r],
                            start=(i == 0), stop=(i == 7), skip_group_check=True,
                        )
                    recip = sbo.tile([128, 512], F32, tag="recip", name=f"rc{h}{b}")
                    nc.vector.reciprocal(recip, ps_s)

                    # last block: AV order ending with the l>=256 chunks so
                    # the left output half can drain while AV finishes — the
                    # pipeline is empty after this block and every exposed
                    # serial step is pure tail latency
                    avorder = [3, 4, 2, 5, 1, 0, 6, 7] if last_blk else AVORDER
                    halves = (((0, 256), 6), ((256, 512), 8)) if last_blk \
                        else (((0, 512), 8),)
                    ps_av = []
                    for cc in range(2):
                        ps_av.append(psav.tile([128, 512], F32, tag=f"av{cc}",
                                               name=f"psav{h}{b}{cc}"))
                    relu_sb = []
                    for cc in range(2):
                        relu_sb.append(sbo.tile([128, 512], F16, tag=f"relu{cc}",
                                                name=f"relu{h}{b}{cc}"))
                    o_sb = sbo.tile([128, 4, 512], F16, tag="osb", name=f"o{h}{b}")
                    done = 0
                    for (l0, l1), steps in halves:
                        for i in range(done, steps):
                            r = avorder[i]
                            lo, w = INTERVALS[r]
                            for cc in range(2):
                                nc.tensor.matmul(
                                    ps_av[cc][:, lo:lo + w],
                                    vT_l[4 * b + r][:, 128 * cc:128 * (cc + 1)], pt[r],
                                    start=(i == 0), stop=(i == steps - 1),
                                    skip_group_check=True,
                                )
                        done = steps
                        lsl = slice(l0, l1)
                        # normalization fused into the relu eviction: keeps
                        # the fp16 tile in range and shortens the output tail
                        for cc in range(2):
                            nc.vector.scalar_tensor_tensor(
                                relu_sb[cc][:, lsl], ps_av[cc][:, lsl], 0.0,
                                recip[:, lsl], op0=Alu.max, op1=Alu.mult,
                            )
                        for oc in range(4):
                            ps_o = pso.tile([128, 512], F32, tag="o",
                                            name=f"pso{h}{b}{oc}{l0}")
                            for cc in range(2):
                                nc.tensor.matmul(
                                    ps_o[:, lsl],
                                    woT_sb[cc][:, 128 * oc:128 * (oc + 1)],
                                    relu_sb[cc][:, lsl],
                                    start=(cc == 0), stop=(cc == 1),
                                )
                            # last block: alternate ACT/DVE so the final
                            # evictions don't serialize on one engine
                            if last_blk and oc % 2:
                                nc.vector.scalar_tensor_tensor(
                                    o_sb[:, oc, lsl], ps_o[:, lsl], 0.0,
                                    recip[:, lsl], op0=Alu.bypass, op1=Alu.bypass,
                                )
                            else:
                                nc.scalar.activation(o_sb[:, oc, lsl], ps_o[:, lsl],
                                                     Act.Copy)
                            if oc % 2:
                                c0 = LH * h + 512 * b
                                nc.sync.dma_start(
                                    out=out_d.ap()[:, oc - 1:oc + 1,
                                                   c0 + lsl.start: c0 + lsl.stop],
                                    in_=o_sb[:, oc - 1:oc + 1, lsl],
                                )

                pending = []
                for b in range(BPH):
                    if b == 0:
                        if h == 0:
                            k_group(0)
                        k_group(1)
                        q_group(0)
                        vT_group(range(0, 8) if h == 0 else range(4, 8))
                    else:
                        k_group(b + 1)
                        q_group(b)
                        vT_group(range(4 * b + 4, 4 * b + 8))
                    pending.append(emit_energy(h, b))
                    if len(pending) > 1:
                        emit_tail(pending.pop(0))
                # flush before the next half's projections overwrite q/k/vT
                for ctx in pending:
                    emit_tail(ctx)
                prev_vT = vT_sb
                if h == 0:
                    # stash the k halo overlap for the next half (SBUF->SBUF
                    # DMA, off-engine); half1's block 0 reads it directly
                    kh_prev = []
                    for cc in range(2):
                        tkh = qkv.tile([128, 512], F16, tag=f"kh{cc}",
                                       name=f"kh{cc}")
                        nc.sync.dma_start(out=tkh,
                                          in_=k_sb[cc][:, LH:LH + 512])
                        kh_prev.append(tkh)
    nc.compile()
    return nc


_NC_CACHE = {}


def _get_nc():
    if "nc" not in _NC_CACHE:
        _NC_CACHE["nc"] = _build_program()
    return _NC_CACHE["nc"]


def _f8():
    try:
        import ml_dtypes
        return ml_dtypes.float8_e4m3
    except ImportError:  # pragma: no cover
        import jax.numpy as jnp
        return jnp.float8_e4m3


def _split8(a):
    f8 = _f8()
    hi = np.asarray(a, np.float32).astype(f8)
    lo = (np.asarray(a, np.float32) - hi.astype(np.float32)).astype(f8)
    return hi, lo


def make_in_maps(x1, mask, Wq, bq, Wk, bk, Wv, bv, Wo, bo):
    x1 = np.asarray(x1, dtype=np.float32).reshape(CIN, L)

    def _pairs(w):
        # (C_out=256, C_in=512) -> [128, 2(hl), 2(p), 2(j), C] DoubleRow pair
        # layout with global c_in = 128*(2p+j) + c_in_within
        ws = np.asarray(w, np.float32) * WSCALE
        hi, lo = _split8(ws.T)          # (512, 256) each
        def lay(a):
            return a.reshape(2, 2, 128, C).transpose(2, 0, 1, 3)
        return np.ascontiguousarray(np.stack([lay(hi), lay(lo)], axis=1))

    wq8 = _pairs(Wq)
    wk8 = _pairs(Wk)
    wv8 = _pairs(Wv)
    woT = (np.asarray(Wo, np.float32).T / WSCALE).astype(np.float16)

    cb32 = np.zeros((128, 264), np.float32)
    cb32[:, 0:2] = (np.asarray(bq, np.float32) * WSCALE).reshape(2, 128).T
    cb32[:, 2:4] = (np.asarray(bk, np.float32) * WSCALE).reshape(2, 128).T
    # padf/padl cols 4:8 filled per core below
    cb32[:, 8:264] = np.broadcast_to(
        (np.asarray(bv, np.float32) * WSCALE).reshape(1, C), (128, C))

    cb16 = np.zeros((128, 1152), np.float16)
    cb16[:, 0:128] = 1.0
    cb16[:, 128:640] = woT[0:128]
    cb16[:, 640:1152] = woT[128:256]

    in_maps = []
    for c in range(NCORES):
        g0 = LC * c - HALF
        x1h = np.zeros((CIN, HALO), np.float32)
        s0, s1 = max(g0, 0), min(g0 + HALO, L)
        x1h[:, s0 - g0:s1 - g0] = x1[:, s0:s1]
        xh, xl = _split8(x1h)
        # [p, c_within, hl, j, pos] with global channel 128*(2p+j) + c_within
        xhl = np.stack([xh.reshape(2, 2, 128, HALO),
                        xl.reshape(2, 2, 128, HALO)], axis=0)
        xhl = np.ascontiguousarray(xhl.transpose(1, 3, 0, 2, 4))
        cb = cb32.copy()
        cb[:, 4:6] = 0.0 if c == 0 else 1.0
        cb[:, 6:8] = 0.0 if c == NCORES - 1 else 1.0
        m = {
            "xhl": xhl,
            "wq": wq8, "wk": wk8, "wv": wv8,
            "cb32": cb, "cb16": cb16,
        }
        in_maps.append(m)
    return in_maps


def postprocess(results, mask, bo):
    # per-core out is [128, 4, LC] c-major; channel g = 128*oc + c
    cols = np.concatenate(
        [np.asarray(results[c]["out"], np.float32).transpose(1, 0, 2)
         .reshape(CIN, LC) for c in range(NCORES)], axis=1)
    out = cols[None] + np.asarray(bo, np.float32)[None, :, None]
    return (out * np.asarray(mask, np.float32)).astype(np.float32)


def kernel(x1, x2, mask, Wq, bq, Wk, bk, Wv, bv, Wo, bo, **_unused):
    from concourse.bass_utils import run_bass_kernel_spmd

    nc = _get_nc()
    in_maps = make_in_maps(x1, mask, Wq, bq, Wk, bk, Wv, bv, Wo, bo)
    res = run_bass_kernel_spmd(nc, in_maps, core_ids=list(range(NCORES)))
    return postprocess(res.results, mask, bo)
